# revision 9
# baseline (speedup 1.0000x reference)
"""AtlasSpecializedLoss on 8 TRN2 NeuronCores — pure data parallel over B.
v4: bf16 compute pipeline + fused DVE ops (tensor_tensor_reduce /
scalar_tensor_tensor), focal computed from softmax (no ptacc path),
per-color center stats via per-channel ttr accumulations (no big
rows/cols tensor_reduce), GpSimd only for odd-offset sobel/edge adds."""

import sys

for _p in ("/opt/trn_rl_repo", "/opt/pypackages"):
    if _p not in sys.path:
        sys.path.append(_p)

import numpy as np

import concourse.bass as bass
import concourse.bacc as bacc
from concourse import mybir
from concourse.tile import TileContext
from concourse.bass_utils import run_bass_kernel_spmd

F32 = mybir.dt.float32
BF16 = mybir.dt.bfloat16
AF = mybir.ActivationFunctionType
OP = mybir.AluOpType
AX = mybir.AxisListType

B, C, H, W = 4096, 10, 30, 30
PIX = H * W
NCOL = C - 1
NCORE = 8
BS = B // NCORE
P = 128
NT = BS // P
CH = 5

# out layout (f32 per row)
O_MPT, O_MCP, O_MC2, O_FOC, O_EDG, O_AFF, O_ROT, O_RFL = 0, 1, 2, 3, 4, 5, 6, 7
O_PC = 8            # 10
O_CNP, O_SYP, O_SXP = 18, 27, 36   # 9 each (colors 1..9) for pred argmax
O_CNT, O_SYT, O_SXT = 45, 54, 63   # 9 each for target
OUTW = 72


def _bc(ap, pos, n):
    dims = list(ap.ap)
    dims.insert(pos + 1, [0, n])
    return bass.AP(tensor=ap.tensor, offset=ap.offset, ap=dims)


USE_TTR = False


def _ttr(v, out, in0, in1, op0, accum):
    """fused (in0 op0 in1) -> sum accumulate into accum [P,1]."""
    if USE_TTR:
        v.tensor_tensor_reduce(out=out, in0=in0, in1=in1, scale=1.0,
                               scalar=0.0, op0=op0, op1=OP.add,
                               accum_out=accum)
    else:
        v.tensor_tensor(out, in0, in1, op0)
        v.tensor_reduce(accum, out, axis=AX.XY if len(out.shape) > 2 else AX.X,
                        op=OP.add)


def build_graph() -> bass.Bass:
    nc = bacc.Bacc()
    pred = nc.declare_dram_parameter("pred", [BS, C * PIX], F32, isOutput=False)
    targ = nc.declare_dram_parameter("targ", [BS, C * PIX], F32, isOutput=False)
    ig = nc.declare_dram_parameter("ig", [BS, C * PIX], F32, isOutput=False)
    theta = nc.declare_dram_parameter("theta", [BS, 6], F32, isOutput=False)
    rot = nc.declare_dram_parameter("rot", [BS, 8], F32, isOutput=False)
    refl = nc.declare_dram_parameter("refl", [BS, 4], F32, isOutput=False)
    xmapd = nc.declare_dram_parameter("xmap", [1, PIX], BF16, isOutput=False)
    ymapd = nc.declare_dram_parameter("ymap", [1, PIX], BF16, isOutput=False)
    out = nc.declare_dram_parameter("out", [BS, OUTW], F32, isOutput=True)

    v = nc.vector
    a = nc.scalar
    g = nc.gpsimd

    with TileContext(nc) as tc:
        # pin combined ln+exp+copy+square act table (avoids per-switch reloads)
        atl = mybir.InstLoadActFuncSet(
            name=nc.get_next_instruction_name(), ins=[], outs=[])
        atl.act_func_set_id = 6
        nc.scalar.add_instruction(atl)
        with (
            tc.tile_pool(name="pr", bufs=1) as prp,
            tc.tile_pool(name="tg", bufs=1) as tgp,
            tc.tile_pool(name="igp", bufs=1) as igp,
            tc.tile_pool(name="eb", bufs=2) as ebp,      # prE / mp rotate here
            tc.tile_pool(name="tgb", bufs=1) as tgbp,
            tc.tile_pool(name="tf", bufs=1) as tfp,      # f32 tree scratch
            tc.tile_pool(name="tb", bufs=1) as tbp,      # bf16 big scratch
            tc.tile_pool(name="sb", bufs=3) as sbp,      # sobel scratch [P,2,900]
            tc.tile_pool(name="sob", bufs=2) as sobp,    # pidx|tidx home
            tc.tile_pool(name="sp", bufs=6) as spp,      # small [P,900] bf16 rotat.
            tc.tile_pool(name="px", bufs=1) as pxp,      # named persistent smalls
            tc.tile_pool(name="outp", bufs=2) as outp,
            tc.tile_pool(name="tiny", bufs=8) as tiny,
            tc.tile_pool(name="cst", bufs=1) as cst,
        ):
            xmap = cst.tile([P, PIX], BF16, tag="xmap")
            src = xmapd[0:1, :]
            nc.sync.dma_start(out=xmap, in_=bass.AP(
                tensor=src.tensor, offset=src.offset, ap=[[0, P], [1, PIX]]))
            ymap = cst.tile([P, PIX], BF16, tag="ymap")
            src = ymapd[0:1, :]
            nc.sync.dma_start(out=ymap, in_=bass.AP(
                tensor=src.tensor, offset=src.offset, ap=[[0, P], [1, PIX]]))

            for t in range(NT):
                r0 = t * P

                pr = prp.tile([P, C, PIX], F32, tag="pr")
                nc.sync.dma_start(
                    out=pr[:, 0:CH, :],
                    in_=pred[r0:r0 + P, :CH * PIX].rearrange("p (c x) -> p c x", c=CH))
                nc.sync.dma_start(
                    out=pr[:, CH:C, :],
                    in_=pred[r0:r0 + P, CH * PIX:].rearrange("p (c x) -> p c x", c=CH))

                ot = outp.tile([P, OUTW], F32, tag="ot")
                g.memset(ot, 0.0)

                # ---- target: load 2 groups, cast to bf16 ----
                tgb = tgbp.tile([P, C, PIX], BF16, tag="tgb")
                for cg in range(2):
                    c0 = cg * CH
                    tg_t = tgp.tile([P, CH, PIX], F32, tag="tg")
                    nc.sync.dma_start(
                        out=tg_t,
                        in_=targ[r0:r0 + P, c0 * PIX:(c0 + CH) * PIX].rearrange(
                            "p (c x) -> p c x", c=CH))
                    v.tensor_scalar(tgb[:, c0:c0 + CH, :], tg_t, 1.0, None, OP.mult)

                # ---- pred softmax pieces ----
                prE = ebp.tile([P, C, PIX], BF16, tag="eb")
                a.activation(prE, pr, AF.Exp)
                prEf = prE.rearrange("p c x -> p (c x)")
                s_b = tbp.tile([P, CH * PIX], BF16, tag="tb")
                v.tensor_add(s_b, prEf[:, 0:4500], prEf[:, 4500:9000])
                v.tensor_add(s_b[:, 0:1800], s_b[:, 0:1800], s_b[:, 1800:3600])
                v.tensor_add(s_b[:, 0:900], s_b[:, 0:900], s_b[:, 900:1800])
                ss = pxp.tile([P, PIX], BF16, tag="ss")
                v.tensor_add(ss, s_b[:, 0:900], s_b[:, 3600:4500])
                lr = pxp.tile([P, PIX], BF16, tag="lr")
                a.activation(lr, ss, AF.Ln)
                rr = pxp.tile([P, PIX], BF16, tag="rr")
                a.activation(rr, lr, AF.Exp, bias=0.0, scale=-1.0)

                # q = sum_c targ*softmax_num = prE[tidx]  (exact: targ one-hot)
                q_b = tbp.tile([P, CH * PIX], BF16, tag="tb")
                tgbf = tgb.rearrange("p c x -> p (c x)")
                q = pxp.tile([P, PIX], BF16, tag="q")
                v.tensor_mul(q_b, tgbf[:, 0:4500], prEf[:, 0:4500])
                v.tensor_add(q_b[:, 0:1800], q_b[:, 0:1800], q_b[:, 1800:3600])
                v.tensor_add(q_b[:, 0:900], q_b[:, 0:900], q_b[:, 900:1800])
                v.tensor_add(q, q_b[:, 0:900], q_b[:, 3600:4500])
                q_b2 = tbp.tile([P, CH * PIX], BF16, tag="tb")
                v.tensor_mul(q_b2, tgbf[:, 4500:9000], prEf[:, 4500:9000])
                v.tensor_add(q_b2[:, 0:1800], q_b2[:, 0:1800], q_b2[:, 1800:3600])
                v.tensor_add(q_b2[:, 0:900], q_b2[:, 0:900], q_b2[:, 900:1800])
                v.tensor_add(q_b2[:, 0:900], q_b2[:, 0:900], q_b2[:, 3600:4500])
                v.tensor_add(q, q, q_b2[:, 0:900])

                # PC[c] = sum_pix softmax_c  (10 fused mult+reduce)
                pc_o = tbp.tile([P, CH * PIX], BF16, tag="tb")
                for c in range(C):
                    _ttr(v, pc_o[:, 0:900], prE[:, c, :], rr, OP.mult,
                         ot[:, O_PC + c:O_PC + c + 1])

                # ---- argmax over channels ----
                mf = tfp.tile([P, CH * PIX], F32, tag="tf")
                prf = pr.rearrange("p c x -> p (c x)")
                v.tensor_max(mf, prf[:, 0:4500], prf[:, 4500:9000])
                v.tensor_max(mf[:, 0:1800], mf[:, 0:1800], mf[:, 1800:3600])
                v.tensor_max(mf[:, 0:900], mf[:, 0:900], mf[:, 900:1800])
                mx = pxp.tile([P, PIX], F32, tag="mx")
                v.tensor_max(mx, mf[:, 0:900], mf[:, 3600:4500])
                mp = ebp.tile([P, C, PIX], BF16, tag="eb")
                v.tensor_tensor(mp, pr, _bc(mx, 0, C), OP.is_equal)

                # ---- copy-match: sum mp*ig  (2 fused mult+reduce) ----
                for cg in range(2):
                    c0 = cg * CH
                    ig_t = igp.tile([P, CH, PIX], F32, tag="ig")
                    nc.sync.dma_start(
                        out=ig_t,
                        in_=ig[r0:r0 + P, c0 * PIX:(c0 + CH) * PIX].rearrange(
                            "p (c x) -> p c x", c=CH))
                    mc_o = tbp.tile([P, CH * PIX], BF16, tag="tb")
                    slot = O_MCP if cg == 0 else O_MC2
                    _ttr(v, mc_o, mp[:, c0:c0 + CH, :].rearrange("p c x -> p (c x)"),
                         ig_t.rearrange("p c x -> p (c x)"), OP.mult,
                         ot[:, slot:slot + 1])

                # ---- pidx / tidx via weighted-channel stt chains ----
                sob = sobp.tile([P, 2, PIX], BF16, tag="sob")
                pidx = sob[:, 0, :]
                tidx = sob[:, 1, :]
                v.tensor_scalar(pidx, mp[:, 1, :], 1.0, None, OP.mult)
                for c in range(2, C):
                    v.scalar_tensor_tensor(pidx, mp[:, c, :], float(c), pidx,
                                           OP.mult, OP.add)
                v.tensor_scalar(tidx, tgb[:, 1, :], 1.0, None, OP.mult)
                for c in range(2, C):
                    v.scalar_tensor_tensor(tidx, tgb[:, c, :], float(c), tidx,
                                           OP.mult, OP.add)

                # exact-match count
                ex_o = spp.tile([P, PIX], BF16, tag="sp")
                _ttr(v, ex_o, pidx, tidx, OP.is_equal,
                     ot[:, O_MPT:O_MPT + 1])

                # ---- per-color center stats: cnt/sy/sx for mp and tgb ----
                st_o = tbp.tile([P, CH * PIX], BF16, tag="tb")
                for c in range(1, C):
                    _ttr(v, st_o[:, 0:900], mp[:, c, :], mp[:, c, :], OP.mult,
                         ot[:, O_CNP + c - 1:O_CNP + c])
                    _ttr(v, st_o[:, 0:900], mp[:, c, :], ymap, OP.mult,
                         ot[:, O_SYP + c - 1:O_SYP + c])
                    _ttr(v, st_o[:, 0:900], mp[:, c, :], xmap, OP.mult,
                         ot[:, O_SXP + c - 1:O_SXP + c])
                for c in range(1, C):
                    _ttr(v, st_o[:, 0:900], tgb[:, c, :], tgb[:, c, :], OP.mult,
                         ot[:, O_CNT + c - 1:O_CNT + c])
                    _ttr(v, st_o[:, 0:900], tgb[:, c, :], ymap, OP.mult,
                         ot[:, O_SYT + c - 1:O_SYT + c])
                    _ttr(v, st_o[:, 0:900], tgb[:, c, :], xmap, OP.mult,
                         ot[:, O_SXT + c - 1:O_SXT + c])

                # ---- edges -> sw ----
                thw = tidx.rearrange("p (h w) -> p h w", w=W)
                ee = spp.tile([P, PIX], BF16, tag="sp")
                v.memset(ee, 0.0)
                dh = spp.tile([P, PIX], BF16, tag="sp")
                v.tensor_tensor(dh[:, :870], tidx[:, 30:], tidx[:, :870],
                                OP.not_equal)
                v.tensor_add(ee[:, 30:], ee[:, 30:], dh[:, :870])
                v.tensor_add(ee[:, :870], ee[:, :870], dh[:, :870])
                dw = spp.tile([P, PIX], BF16, tag="sp")
                dwv = dw[:, :870].rearrange("p (h w) -> p h w", w=29)
                v.tensor_tensor(dwv, thw[:, :, 1:], thw[:, :, :29], OP.not_equal)
                eehw = ee.rearrange("p (h w) -> p h w", w=W)
                v.tensor_add(eehw[:, :, 1:], eehw[:, :, 1:], dwv)
                v.tensor_add(eehw[:, :, :29], eehw[:, :, :29], dwv)
                sw = pxp.tile([P, PIX], BF16, tag="sw")
                v.tensor_scalar(sw, ee, 0.0, None, OP.is_gt)
                a.activation(sw, sw, AF.Copy, bias=1.0, scale=0.5)

                # ---- focal:  sum (1-pt)^1.4 * ln(pt) * sw   (host negates) ----
                pt = pxp.tile([P, PIX], BF16, tag="pt")
                v.tensor_mul(pt, q, rr)
                ceb = spp.tile([P, PIX], BF16, tag="sp")
                a.activation(ceb, pt, AF.Ln)
                tm = spp.tile([P, PIX], BF16, tag="sp")
                v.tensor_scalar(tm, pt, -1.0, 1.0, OP.mult, OP.add)
                v.tensor_scalar_max(tm, tm, 1e-30)
                a.activation(tm, tm, AF.Ln)
                vb = spp.tile([P, PIX], BF16, tag="sp")
                a.activation(vb, tm, AF.Exp, bias=0.0, scale=1.4)
                wf = spp.tile([P, PIX], BF16, tag="sp")
                v.tensor_mul(wf, vb, ceb)
                foc_o = spp.tile([P, PIX], BF16, tag="sp")
                _ttr(v, foc_o, wf, sw, OP.mult, ot[:, O_FOC:O_FOC + 1])

                # ---- sobel on [pidx|tidx] jointly ----
                S = sbp.tile([P, 2, PIX], BF16, tag="sb")
                v.tensor_scalar(S, sob, 2.0, None, OP.mult)
                v.tensor_add(S[:, :, 30:], S[:, :, 30:], sob[:, :, :870])
                v.tensor_add(S[:, :, :870], S[:, :, :870], sob[:, :, 30:])
                EX = sbp.tile([P, 2, PIX], BF16, tag="sb")
                vS = S.rearrange("p c (h w) -> p c h w", w=W)
                vE = EX.rearrange("p c (h w) -> p c h w", w=W)
                v.tensor_scalar(vE[:, :, :, 0:1], vS[:, :, :, 1:2], 1.0, None, OP.mult)
                v.tensor_scalar(vE[:, :, :, 29:30], vS[:, :, :, 28:29], -1.0, None, OP.mult)
                v.tensor_sub(vE[:, :, :, 1:29], vS[:, :, :, 2:], vS[:, :, :, :28])
                T = sbp.tile([P, 2, PIX], BF16, tag="sb")
                v.tensor_scalar(T, sob, 2.0, None, OP.mult)
                vI = sob.rearrange("p c (h w) -> p c h w", w=W)
                vT = T.rearrange("p c (h w) -> p c h w", w=W)
                v.tensor_add(vT[:, :, :, 1:], vT[:, :, :, 1:], vI[:, :, :, :29])
                v.tensor_add(vT[:, :, :, :29], vT[:, :, :, :29], vI[:, :, :, 1:])
                EY = sbp.tile([P, 2, PIX], BF16, tag="sb")
                v.tensor_scalar(EY[:, :, :30], T[:, :, 30:60], 1.0, None, OP.mult)
                v.tensor_scalar(EY[:, :, 870:], T[:, :, 840:870], -1.0, None, OP.mult)
                v.tensor_sub(EY[:, :, 30:870], T[:, :, 60:], T[:, :, :840])
                v.tensor_mul(EX, EX, EX)
                v.tensor_mul(EY, EY, EY)
                v.tensor_add(EX, EX, EY)
                v.tensor_scalar_max(EX, EX, 1e-30)
                a.activation(EX, EX, AF.Ln)
                a.activation(EX, EX, AF.Exp, bias=0.0, scale=0.5)
                dm = spp.tile([P, PIX], BF16, tag="sp")
                v.tensor_sub(dm, EX[:, 0, :], EX[:, 1, :])
                a.activation(dm, dm, AF.Square, accum_out=ot[:, O_EDG:O_EDG + 1])

                # ---- theta / entropies (same as v3) ----
                th = tiny.tile([P, 6], F32, tag="th")
                nc.sync.dma_start(out=th, in_=theta[r0:r0 + P, :])
                a.square(th, th)
                ssum = tiny.tile([P, 2], F32, tag="ssum")
                v.tensor_reduce(ssum[:, 0:1],
                                th.rearrange("p (r k) -> p r k", k=3)[:, :, 0:2],
                                axis=AX.XY, op=OP.add)
                v.tensor_reduce(ssum[:, 1:2],
                                th.rearrange("p (r k) -> p r k", k=3)[:, :, 2:3],
                                axis=AX.XY, op=OP.add)
                v.tensor_scalar_max(ssum, ssum, 1e-30)
                a.activation(ssum, ssum, AF.Ln)
                a.activation(ssum, ssum, AF.Exp, bias=0.0, scale=0.5)
                qq = tiny.tile([P, 1], F32, tag="q1")
                a.activation(qq, ssum[:, 1:2], AF.Copy, bias=0.0, scale=0.1)
                v.tensor_add(ot[:, O_AFF:O_AFF + 1], ssum[:, 0:1], qq)

                def entropy(src2, n, dst, tagp):
                    lgt = tiny.tile([P, n], F32, tag=tagp)
                    nc.sync.dma_start(out=lgt, in_=src2[r0:r0 + P, :])
                    m8 = tiny.tile([P, 1], F32, tag=tagp + "m")
                    v.tensor_reduce(m8, lgt, axis=AX.X, op=OP.max)
                    nm = tiny.tile([P, 1], F32, tag=tagp + "n")
                    a.activation(nm, m8, AF.Copy, bias=0.0, scale=-1.0)
                    z8 = tiny.tile([P, n], F32, tag=tagp + "z")
                    v.tensor_scalar(z8, lgt, nm, None, OP.add)
                    e8 = tiny.tile([P, n], F32, tag=tagp + "e")
                    a.activation(e8, lgt, AF.Exp, bias=nm)
                    s8 = tiny.tile([P, 1], F32, tag=tagp + "s")
                    v.tensor_reduce(s8, e8, axis=AX.X, op=OP.add)
                    dot = tiny.tile([P, 1], F32, tag=tagp + "d")
                    dsk = tiny.tile([P, n], F32, tag=tagp + "k")
                    v.tensor_mul(dsk, e8, z8)
                    v.tensor_reduce(dot, dsk, axis=AX.X, op=OP.add)
                    r8 = tiny.tile([P, 1], F32, tag=tagp + "r")
                    v.reciprocal(r8, s8)
                    v.tensor_mul(dot, dot, r8)
                    a.activation(s8, s8, AF.Ln)
                    v.tensor_sub(dst, s8, dot)

                entropy(rot, 8, ot[:, O_ROT:O_ROT + 1], "ro")
                entropy(refl, 4, ot[:, O_RFL:O_RFL + 1], "rf")

                nc.sync.dma_start(out=out[r0:r0 + P, :], in_=ot)
    nc.finalize()
    return nc


_GRAPH = None


def _get_graph():
    global _GRAPH
    if _GRAPH is None:
        _GRAPH = build_graph()
    return _GRAPH


def run_device(inputs: dict, trace: bool = False):
    pred = np.asarray(inputs["pred_output"], np.float32).reshape(B, C * PIX)
    targ = np.asarray(inputs["target_output"], np.float32).reshape(B, C * PIX)
    igrid = np.asarray(inputs["input_grid"], np.float32).reshape(B, C * PIX)
    theta = np.asarray(inputs["theta"], np.float32).reshape(B, 6)
    rot = np.asarray(inputs["rotation_logits"], np.float32).reshape(B, 8)
    refl = np.asarray(inputs["reflection_logits"], np.float32).reshape(B, 4)

    import ml_dtypes
    xmap = np.tile(np.arange(W, dtype=np.float32), H).reshape(1, PIX)
    ymap = np.repeat(np.arange(H, dtype=np.float32), W).reshape(1, PIX)
    xmap = xmap.astype(ml_dtypes.bfloat16)
    ymap = ymap.astype(ml_dtypes.bfloat16)

    in_maps = []
    for i in range(NCORE):
        s = slice(i * BS, (i + 1) * BS)
        in_maps.append({
            "pred": np.ascontiguousarray(pred[s]),
            "targ": np.ascontiguousarray(targ[s]),
            "ig": np.ascontiguousarray(igrid[s]),
            "theta": np.ascontiguousarray(theta[s]),
            "rot": np.ascontiguousarray(rot[s]),
            "refl": np.ascontiguousarray(refl[s]),
            "xmap": xmap,
            "ymap": ymap,
        })
    res = run_bass_kernel_spmd(_get_graph(), in_maps, core_ids=list(range(NCORE)),
                               trace=trace)
    outs = np.concatenate([r["out"] for r in res.results], axis=0)
    return outs, res


def assemble(outs: np.ndarray) -> np.ndarray:
    o = outs.astype(np.float64)
    npix = float(B * PIX)
    match_pt = o[:, O_MPT]
    match_cp = o[:, O_MCP] + o[:, O_MC2]
    spatial_focal = -o[:, O_FOC].sum() / npix
    exact = match_pt == PIX
    exact_count = exact.sum()
    exact_bonus = -exact.mean() * 7.0
    transform = (match_cp == PIX).mean() * 0.2
    affine = o[:, O_AFF].mean() * 0.4
    rotation = o[:, O_ROT].mean() * 0.3
    reflection = o[:, O_RFL].mean() * 0.3
    edge = o[:, O_EDG].sum() / npix * 0.3

    pc = o[:, O_PC:O_PC + C]
    cnt_p = o[:, O_CNP:O_CNP + NCOL]
    sy_p = o[:, O_SYP:O_SYP + NCOL]
    sx_p = o[:, O_SXP:O_SXP + NCOL]
    cnt_t = o[:, O_CNT:O_CNT + NCOL]
    sy_t = o[:, O_SYT:O_SYT + NCOL]
    sx_t = o[:, O_SXT:O_SXT + NCOL]

    tc0 = PIX - cnt_t.sum(1, keepdims=True)
    tc_full = np.concatenate([tc0, cnt_t], axis=1)
    pcn = pc / (pc.sum(1, keepdims=True) + 1e-8)
    tcn = tc_full / (tc_full.sum(1, keepdims=True) + 1e-8)
    cbal = ((pcn - tcn) ** 2).mean() * 0.2

    def centers(cnt, sy, sx):
        d = np.maximum(cnt, 1.0)
        return sy / d, sx / d, cnt > 0

    cyp, cxp, prp = centers(cnt_p, sy_p, sx_p)
    cyt, cxt, prt = centers(cnt_t, sy_t, sx_t)
    PI, PJ = np.triu_indices(NCOL, 1)
    NP = PI.shape[0]

    def compact(cy, cx, pres):
        d = np.sqrt((cy[:, PI] - cy[:, PJ]) ** 2 + (cx[:, PI] - cx[:, PJ]) ** 2)
        vv = pres[:, PI] & pres[:, PJ]
        rank = np.cumsum(vv, axis=1) - 1
        slot = np.where(vv, rank, NP)
        comp = np.zeros((B, NP + 1))
        np.put_along_axis(comp, slot, d, axis=1)
        return comp[:, :NP], vv.sum(1)

    dpc, n_p = compact(cyp, cxp, prp)
    dtc, n_t = compact(cyt, cxt, prt)
    m = np.minimum(n_p, n_t)
    use = np.arange(NP)[None, :] < m[:, None]
    sq = (((dpc - dtc) ** 2) * use).sum(1)
    geo_b = np.where(m > 0, sq / np.maximum(m, 1), 0.0)
    geo = geo_b.sum() / B * 0.5

    total = (spatial_focal + transform + affine + rotation + reflection
             + geo + edge + cbal + exact_bonus)
    return np.array([total, spatial_focal, transform, exact_bonus, exact_count,
                     affine, rotation, reflection, geo, edge, cbal], np.float32)


def kernel(**inputs) -> np.ndarray:
    outs, _ = run_device(inputs, trace=False)
    return assemble(outs)


# revision 14
# speedup vs baseline: 1.3083x; 1.3083x over previous
"""AtlasSpecializedLoss on 8 TRN2 NeuronCores — pure data parallel over B.
v4: bf16 compute pipeline + fused DVE ops (tensor_tensor_reduce /
scalar_tensor_tensor), focal computed from softmax (no ptacc path),
per-color center stats via per-channel ttr accumulations (no big
rows/cols tensor_reduce), GpSimd only for odd-offset sobel/edge adds."""

import sys

for _p in ("/opt/trn_rl_repo", "/opt/pypackages"):
    if _p not in sys.path:
        sys.path.append(_p)

import numpy as np

import concourse.bass as bass
import concourse.bacc as bacc
from concourse import mybir
from concourse.tile import TileContext
from concourse.bass_utils import run_bass_kernel_spmd

F32 = mybir.dt.float32
BF16 = mybir.dt.bfloat16
AF = mybir.ActivationFunctionType
OP = mybir.AluOpType
AX = mybir.AxisListType

B, C, H, W = 4096, 10, 30, 30
PIX = H * W
NCOL = C - 1
NCORE = 8
BS = B // NCORE
P = 128
NT = BS // P
CH = 5

# out layout (f32 per row)
O_MPT, O_MCP, O_MC2, O_FOC, O_EDG, O_AFF, O_ROT, O_RFL = 0, 1, 2, 3, 4, 5, 6, 7
O_PC = 8            # 10
O_CNP, O_SYP, O_SXP = 18, 27, 36   # 9 each (colors 1..9) for pred argmax
O_CNT, O_SYT, O_SXT = 45, 54, 63   # 9 each for target
OUTW = 72


def _bc(ap, pos, n):
    dims = list(ap.ap)
    dims.insert(pos + 1, [0, n])
    return bass.AP(tensor=ap.tensor, offset=ap.offset, ap=dims)


def _ttr(v, out, in0, in1, op0, accum):
    """fused (in0 op0 in1) -> sum into accum [P,1], via scalar_tensor_tensor
    (out = (in0 op0' scalar) op1 in1 with accum_out)."""
    if op0 == OP.mult:
        v.scalar_tensor_tensor(out, in0, 1.0, in1, OP.mult, OP.mult,
                               accum_out=accum)
    else:
        v.scalar_tensor_tensor(out, in0, 0.0, in1, OP.bypass, op0,
                               accum_out=accum)


def build_graph() -> bass.Bass:
    nc = bacc.Bacc()
    pred = nc.declare_dram_parameter("pred", [BS, C * PIX], F32, isOutput=False)
    targ = nc.declare_dram_parameter("targ", [BS, C * PIX], F32, isOutput=False)
    ig = nc.declare_dram_parameter("ig", [BS, C * PIX], F32, isOutput=False)
    theta = nc.declare_dram_parameter("theta", [BS, 6], F32, isOutput=False)
    rot = nc.declare_dram_parameter("rot", [BS, 8], F32, isOutput=False)
    refl = nc.declare_dram_parameter("refl", [BS, 4], F32, isOutput=False)
    xmapd = nc.declare_dram_parameter("xmap", [1, PIX], BF16, isOutput=False)
    ymapd = nc.declare_dram_parameter("ymap", [1, PIX], BF16, isOutput=False)
    out = nc.declare_dram_parameter("out", [BS, OUTW], F32, isOutput=True)

    v = nc.vector
    a = nc.scalar
    g = nc.gpsimd

    with TileContext(nc) as tc:
        # pin combined ln+exp+copy+square act table (avoids per-switch reloads)
        atl = mybir.InstLoadActFuncSet(
            name=nc.get_next_instruction_name(), ins=[], outs=[])
        atl.act_func_set_id = 6
        nc.scalar.add_instruction(atl)
        with (
            tc.tile_pool(name="pr", bufs=1) as prp,
            tc.tile_pool(name="tg", bufs=1) as tgp,
            tc.tile_pool(name="igp", bufs=1) as igp,
            tc.tile_pool(name="eb", bufs=2) as ebp,      # prE / mp rotate here
            tc.tile_pool(name="tgb", bufs=1) as tgbp,
            tc.tile_pool(name="tf", bufs=1) as tfp,      # f32 tree scratch
            tc.tile_pool(name="tb", bufs=1) as tbp,      # bf16 big scratch
            tc.tile_pool(name="sb", bufs=3) as sbp,      # sobel scratch [P,2,900]
            tc.tile_pool(name="sob", bufs=2) as sobp,    # pidx|tidx home
            tc.tile_pool(name="sp", bufs=6) as spp,      # small [P,900] bf16 rotat.
            tc.tile_pool(name="px", bufs=1) as pxp,      # named persistent smalls
            tc.tile_pool(name="outp", bufs=2) as outp,
            tc.tile_pool(name="tiny", bufs=8) as tiny,
            tc.tile_pool(name="cst", bufs=1) as cst,
        ):
            xmap = cst.tile([P, PIX], BF16, tag="xmap")
            src = xmapd[0:1, :]
            nc.sync.dma_start(out=xmap, in_=bass.AP(
                tensor=src.tensor, offset=src.offset, ap=[[0, P], [1, PIX]]))
            ymap = cst.tile([P, PIX], BF16, tag="ymap")
            src = ymapd[0:1, :]
            nc.sync.dma_start(out=ymap, in_=bass.AP(
                tensor=src.tensor, offset=src.offset, ap=[[0, P], [1, PIX]]))

            for t in range(NT):
                r0 = t * P

                pr = prp.tile([P, C, PIX], F32, tag="pr")
                nc.sync.dma_start(
                    out=pr[:, 0:CH, :],
                    in_=pred[r0:r0 + P, :CH * PIX].rearrange("p (c x) -> p c x", c=CH))
                nc.sync.dma_start(
                    out=pr[:, CH:C, :],
                    in_=pred[r0:r0 + P, CH * PIX:].rearrange("p (c x) -> p c x", c=CH))

                ot = outp.tile([P, OUTW], F32, tag="ot")
                g.memset(ot, 0.0)

                # ---- target: load 2 groups, cast to bf16 ----
                tgb = tgbp.tile([P, C, PIX], BF16, tag="tgb")
                for cg in range(2):
                    c0 = cg * CH
                    tg_t = tgp.tile([P, CH, PIX], F32, tag="tg")
                    nc.sync.dma_start(
                        out=tg_t,
                        in_=targ[r0:r0 + P, c0 * PIX:(c0 + CH) * PIX].rearrange(
                            "p (c x) -> p c x", c=CH))
                    v.tensor_scalar(tgb[:, c0:c0 + CH, :], tg_t, 1.0, None, OP.mult)

                # ---- pred softmax pieces ----
                prE = ebp.tile([P, C, PIX], BF16, tag="eb")
                a.activation(prE, pr, AF.Exp)
                prEf = prE.rearrange("p c x -> p (c x)")
                s_b = tbp.tile([P, CH * PIX], BF16, tag="tb")
                v.tensor_add(s_b, prEf[:, 0:4500], prEf[:, 4500:9000])
                v.tensor_add(s_b[:, 0:1800], s_b[:, 0:1800], s_b[:, 1800:3600])
                v.tensor_add(s_b[:, 0:900], s_b[:, 0:900], s_b[:, 900:1800])
                ss = pxp.tile([P, PIX], BF16, tag="ss")
                v.tensor_add(ss, s_b[:, 0:900], s_b[:, 3600:4500])
                lr = pxp.tile([P, PIX], BF16, tag="lr")
                a.activation(lr, ss, AF.Ln)
                rr = pxp.tile([P, PIX], BF16, tag="rr")
                a.activation(rr, lr, AF.Exp, bias=0.0, scale=-1.0)

                # q = sum_c targ*softmax_num = prE[tidx]  (exact: targ one-hot)
                q_b = tbp.tile([P, CH * PIX], BF16, tag="tb")
                tgbf = tgb.rearrange("p c x -> p (c x)")
                q = pxp.tile([P, PIX], BF16, tag="q")
                v.tensor_mul(q_b, tgbf[:, 0:4500], prEf[:, 0:4500])
                v.tensor_add(q_b[:, 0:1800], q_b[:, 0:1800], q_b[:, 1800:3600])
                v.tensor_add(q_b[:, 0:900], q_b[:, 0:900], q_b[:, 900:1800])
                v.tensor_add(q, q_b[:, 0:900], q_b[:, 3600:4500])
                q_b2 = tbp.tile([P, CH * PIX], BF16, tag="tb")
                v.tensor_mul(q_b2, tgbf[:, 4500:9000], prEf[:, 4500:9000])
                v.tensor_add(q_b2[:, 0:1800], q_b2[:, 0:1800], q_b2[:, 1800:3600])
                v.tensor_add(q_b2[:, 0:900], q_b2[:, 0:900], q_b2[:, 900:1800])
                v.tensor_add(q_b2[:, 0:900], q_b2[:, 0:900], q_b2[:, 3600:4500])
                v.tensor_add(q, q, q_b2[:, 0:900])

                # PC[c] = sum_pix softmax_c  (10 fused mult+reduce)
                pc_o = tbp.tile([P, CH * PIX], BF16, tag="tb")
                for c in range(C):
                    _ttr(v, pc_o[:, 0:900], prE[:, c, :], rr, OP.mult,
                         ot[:, O_PC + c:O_PC + c + 1])

                # ---- argmax over channels ----
                mf = tfp.tile([P, CH * PIX], F32, tag="tf")
                prf = pr.rearrange("p c x -> p (c x)")
                v.tensor_max(mf, prf[:, 0:4500], prf[:, 4500:9000])
                v.tensor_max(mf[:, 0:1800], mf[:, 0:1800], mf[:, 1800:3600])
                v.tensor_max(mf[:, 0:900], mf[:, 0:900], mf[:, 900:1800])
                mx = pxp.tile([P, PIX], F32, tag="mx")
                v.tensor_max(mx, mf[:, 0:900], mf[:, 3600:4500])
                mp = ebp.tile([P, C, PIX], BF16, tag="eb")
                v.tensor_tensor(mp, pr, _bc(mx, 0, C), OP.is_equal)

                # ---- copy-match: sum mp*ig  (2 fused mult+reduce) ----
                for cg in range(2):
                    c0 = cg * CH
                    ig_t = igp.tile([P, CH, PIX], F32, tag="ig")
                    nc.sync.dma_start(
                        out=ig_t,
                        in_=ig[r0:r0 + P, c0 * PIX:(c0 + CH) * PIX].rearrange(
                            "p (c x) -> p c x", c=CH))
                    mc_o = tbp.tile([P, CH * PIX], BF16, tag="tb")
                    slot = O_MCP if cg == 0 else O_MC2
                    _ttr(v, mc_o, mp[:, c0:c0 + CH, :].rearrange("p c x -> p (c x)"),
                         ig_t.rearrange("p c x -> p (c x)"), OP.mult,
                         ot[:, slot:slot + 1])

                # ---- pidx / tidx via weighted-channel stt chains ----
                sob = sobp.tile([P, 2, PIX], BF16, tag="sob")
                pidx = sob[:, 0, :]
                tidx = sob[:, 1, :]
                v.tensor_scalar(pidx, mp[:, 1, :], 1.0, None, OP.mult)
                for c in range(2, C):
                    v.scalar_tensor_tensor(pidx, mp[:, c, :], float(c), pidx,
                                           OP.mult, OP.add)
                v.tensor_scalar(tidx, tgb[:, 1, :], 1.0, None, OP.mult)
                for c in range(2, C):
                    v.scalar_tensor_tensor(tidx, tgb[:, c, :], float(c), tidx,
                                           OP.mult, OP.add)

                # exact-match count
                ex_o = spp.tile([P, PIX], BF16, tag="sp")
                _ttr(v, ex_o, pidx, tidx, OP.is_equal,
                     ot[:, O_MPT:O_MPT + 1])

                # ---- per-color center stats: cnt/sy/sx for mp and tgb ----
                st_o = tbp.tile([P, CH * PIX], BF16, tag="tb")
                for c in range(1, C):
                    v.tensor_scalar(st_o[:, 0:900], mp[:, c, :], 1.0, 0.0,
                                    OP.mult, OP.add,
                                    accum_out=ot[:, O_CNP + c - 1:O_CNP + c])
                    _ttr(v, st_o[:, 0:900], mp[:, c, :], ymap, OP.mult,
                         ot[:, O_SYP + c - 1:O_SYP + c])
                    _ttr(v, st_o[:, 0:900], mp[:, c, :], xmap, OP.mult,
                         ot[:, O_SXP + c - 1:O_SXP + c])
                for c in range(1, C):
                    v.tensor_scalar(st_o[:, 0:900], tgb[:, c, :], 1.0, 0.0,
                                    OP.mult, OP.add,
                                    accum_out=ot[:, O_CNT + c - 1:O_CNT + c])
                    _ttr(v, st_o[:, 0:900], tgb[:, c, :], ymap, OP.mult,
                         ot[:, O_SYT + c - 1:O_SYT + c])
                    _ttr(v, st_o[:, 0:900], tgb[:, c, :], xmap, OP.mult,
                         ot[:, O_SXT + c - 1:O_SXT + c])

                # ---- edges -> sw ----
                thw = tidx.rearrange("p (h w) -> p h w", w=W)
                ee = spp.tile([P, PIX], BF16, tag="sp")
                v.memset(ee, 0.0)
                dh = spp.tile([P, PIX], BF16, tag="sp")
                v.tensor_tensor(dh[:, :870], tidx[:, 30:], tidx[:, :870],
                                OP.not_equal)
                v.tensor_add(ee[:, 30:], ee[:, 30:], dh[:, :870])
                v.tensor_add(ee[:, :870], ee[:, :870], dh[:, :870])
                dw = spp.tile([P, PIX], BF16, tag="sp")
                dwv = dw[:, :870].rearrange("p (h w) -> p h w", w=29)
                v.tensor_tensor(dwv, thw[:, :, 1:], thw[:, :, :29], OP.not_equal)
                eehw = ee.rearrange("p (h w) -> p h w", w=W)
                v.tensor_add(eehw[:, :, 1:], eehw[:, :, 1:], dwv)
                v.tensor_add(eehw[:, :, :29], eehw[:, :, :29], dwv)
                sw = pxp.tile([P, PIX], BF16, tag="sw")
                v.tensor_scalar(sw, ee, 0.0, None, OP.is_gt)
                a.activation(sw, sw, AF.Copy, bias=1.0, scale=0.5)

                # ---- focal:  sum (1-pt)^1.4 * ln(pt) * sw   (host negates) ----
                pt = pxp.tile([P, PIX], BF16, tag="pt")
                v.tensor_mul(pt, q, rr)
                ceb = spp.tile([P, PIX], BF16, tag="sp")
                a.activation(ceb, pt, AF.Ln)
                tm = spp.tile([P, PIX], BF16, tag="sp")
                v.tensor_scalar(tm, pt, -1.0, 1.0, OP.mult, OP.add)
                v.tensor_scalar_max(tm, tm, 1e-30)
                a.activation(tm, tm, AF.Ln)
                vb = spp.tile([P, PIX], BF16, tag="sp")
                a.activation(vb, tm, AF.Exp, bias=0.0, scale=1.4)
                wf = spp.tile([P, PIX], BF16, tag="sp")
                v.tensor_mul(wf, vb, ceb)
                foc_o = spp.tile([P, PIX], BF16, tag="sp")
                _ttr(v, foc_o, wf, sw, OP.mult, ot[:, O_FOC:O_FOC + 1])

                # ---- sobel on [pidx|tidx] jointly ----
                S = sbp.tile([P, 2, PIX], BF16, tag="sb")
                v.tensor_scalar(S, sob, 2.0, None, OP.mult)
                v.tensor_add(S[:, :, 30:], S[:, :, 30:], sob[:, :, :870])
                v.tensor_add(S[:, :, :870], S[:, :, :870], sob[:, :, 30:])
                EX = sbp.tile([P, 2, PIX], BF16, tag="sb")
                vS = S.rearrange("p c (h w) -> p c h w", w=W)
                vE = EX.rearrange("p c (h w) -> p c h w", w=W)
                v.tensor_scalar(vE[:, :, :, 0:1], vS[:, :, :, 1:2], 1.0, None, OP.mult)
                v.tensor_scalar(vE[:, :, :, 29:30], vS[:, :, :, 28:29], -1.0, None, OP.mult)
                v.tensor_sub(vE[:, :, :, 1:29], vS[:, :, :, 2:], vS[:, :, :, :28])
                T = sbp.tile([P, 2, PIX], BF16, tag="sb")
                v.tensor_scalar(T, sob, 2.0, None, OP.mult)
                vI = sob.rearrange("p c (h w) -> p c h w", w=W)
                vT = T.rearrange("p c (h w) -> p c h w", w=W)
                v.tensor_add(vT[:, :, :, 1:], vT[:, :, :, 1:], vI[:, :, :, :29])
                v.tensor_add(vT[:, :, :, :29], vT[:, :, :, :29], vI[:, :, :, 1:])
                EY = sbp.tile([P, 2, PIX], BF16, tag="sb")
                v.tensor_scalar(EY[:, :, :30], T[:, :, 30:60], 1.0, None, OP.mult)
                v.tensor_scalar(EY[:, :, 870:], T[:, :, 840:870], -1.0, None, OP.mult)
                v.tensor_sub(EY[:, :, 30:870], T[:, :, 60:], T[:, :, :840])
                v.tensor_mul(EX, EX, EX)
                v.tensor_mul(EY, EY, EY)
                v.tensor_add(EX, EX, EY)
                v.tensor_scalar_max(EX, EX, 1e-30)
                a.activation(EX, EX, AF.Ln)
                a.activation(EX, EX, AF.Exp, bias=0.0, scale=0.5)
                dm = spp.tile([P, PIX], BF16, tag="sp")
                v.tensor_sub(dm, EX[:, 0, :], EX[:, 1, :])
                a.activation(dm, dm, AF.Square, accum_out=ot[:, O_EDG:O_EDG + 1])

                # ---- theta / entropies (same as v3) ----
                th = tiny.tile([P, 6], F32, tag="th")
                nc.sync.dma_start(out=th, in_=theta[r0:r0 + P, :])
                a.square(th, th)
                ssum = tiny.tile([P, 2], F32, tag="ssum")
                v.tensor_reduce(ssum[:, 0:1],
                                th.rearrange("p (r k) -> p r k", k=3)[:, :, 0:2],
                                axis=AX.XY, op=OP.add)
                v.tensor_reduce(ssum[:, 1:2],
                                th.rearrange("p (r k) -> p r k", k=3)[:, :, 2:3],
                                axis=AX.XY, op=OP.add)
                v.tensor_scalar_max(ssum, ssum, 1e-30)
                a.activation(ssum, ssum, AF.Ln)
                a.activation(ssum, ssum, AF.Exp, bias=0.0, scale=0.5)
                qq = tiny.tile([P, 1], F32, tag="q1")
                a.activation(qq, ssum[:, 1:2], AF.Copy, bias=0.0, scale=0.1)
                v.tensor_add(ot[:, O_AFF:O_AFF + 1], ssum[:, 0:1], qq)

                def entropy(src2, n, dst, tagp):
                    lgt = tiny.tile([P, n], F32, tag=tagp)
                    nc.sync.dma_start(out=lgt, in_=src2[r0:r0 + P, :])
                    m8 = tiny.tile([P, 1], F32, tag=tagp + "m")
                    v.tensor_reduce(m8, lgt, axis=AX.X, op=OP.max)
                    nm = tiny.tile([P, 1], F32, tag=tagp + "n")
                    a.activation(nm, m8, AF.Copy, bias=0.0, scale=-1.0)
                    z8 = tiny.tile([P, n], F32, tag=tagp + "z")
                    v.tensor_scalar(z8, lgt, nm, None, OP.add)
                    e8 = tiny.tile([P, n], F32, tag=tagp + "e")
                    a.activation(e8, lgt, AF.Exp, bias=nm)
                    s8 = tiny.tile([P, 1], F32, tag=tagp + "s")
                    v.tensor_reduce(s8, e8, axis=AX.X, op=OP.add)
                    dot = tiny.tile([P, 1], F32, tag=tagp + "d")
                    dsk = tiny.tile([P, n], F32, tag=tagp + "k")
                    v.tensor_mul(dsk, e8, z8)
                    v.tensor_reduce(dot, dsk, axis=AX.X, op=OP.add)
                    r8 = tiny.tile([P, 1], F32, tag=tagp + "r")
                    v.reciprocal(r8, s8)
                    v.tensor_mul(dot, dot, r8)
                    a.activation(s8, s8, AF.Ln)
                    v.tensor_sub(dst, s8, dot)

                entropy(rot, 8, ot[:, O_ROT:O_ROT + 1], "ro")
                entropy(refl, 4, ot[:, O_RFL:O_RFL + 1], "rf")

                nc.sync.dma_start(out=out[r0:r0 + P, :], in_=ot)
    nc.finalize()
    return nc


_GRAPH = None


def _get_graph():
    global _GRAPH
    if _GRAPH is None:
        _GRAPH = build_graph()
    return _GRAPH


def run_device(inputs: dict, trace: bool = False):
    pred = np.asarray(inputs["pred_output"], np.float32).reshape(B, C * PIX)
    targ = np.asarray(inputs["target_output"], np.float32).reshape(B, C * PIX)
    igrid = np.asarray(inputs["input_grid"], np.float32).reshape(B, C * PIX)
    theta = np.asarray(inputs["theta"], np.float32).reshape(B, 6)
    rot = np.asarray(inputs["rotation_logits"], np.float32).reshape(B, 8)
    refl = np.asarray(inputs["reflection_logits"], np.float32).reshape(B, 4)

    import ml_dtypes
    xmap = np.tile(np.arange(W, dtype=np.float32), H).reshape(1, PIX)
    ymap = np.repeat(np.arange(H, dtype=np.float32), W).reshape(1, PIX)
    xmap = xmap.astype(ml_dtypes.bfloat16)
    ymap = ymap.astype(ml_dtypes.bfloat16)

    in_maps = []
    for i in range(NCORE):
        s = slice(i * BS, (i + 1) * BS)
        in_maps.append({
            "pred": np.ascontiguousarray(pred[s]),
            "targ": np.ascontiguousarray(targ[s]),
            "ig": np.ascontiguousarray(igrid[s]),
            "theta": np.ascontiguousarray(theta[s]),
            "rot": np.ascontiguousarray(rot[s]),
            "refl": np.ascontiguousarray(refl[s]),
            "xmap": xmap,
            "ymap": ymap,
        })
    res = run_bass_kernel_spmd(_get_graph(), in_maps, core_ids=list(range(NCORE)),
                               trace=trace)
    outs = np.concatenate([r["out"] for r in res.results], axis=0)
    return outs, res


def assemble(outs: np.ndarray) -> np.ndarray:
    o = outs.astype(np.float64)
    npix = float(B * PIX)
    match_pt = o[:, O_MPT]
    match_cp = o[:, O_MCP] + o[:, O_MC2]
    spatial_focal = -o[:, O_FOC].sum() / npix
    exact = match_pt == PIX
    exact_count = exact.sum()
    exact_bonus = -exact.mean() * 7.0
    transform = (match_cp == PIX).mean() * 0.2
    affine = o[:, O_AFF].mean() * 0.4
    rotation = o[:, O_ROT].mean() * 0.3
    reflection = o[:, O_RFL].mean() * 0.3
    edge = o[:, O_EDG].sum() / npix * 0.3

    pc = o[:, O_PC:O_PC + C]
    cnt_p = o[:, O_CNP:O_CNP + NCOL]
    sy_p = o[:, O_SYP:O_SYP + NCOL]
    sx_p = o[:, O_SXP:O_SXP + NCOL]
    cnt_t = o[:, O_CNT:O_CNT + NCOL]
    sy_t = o[:, O_SYT:O_SYT + NCOL]
    sx_t = o[:, O_SXT:O_SXT + NCOL]

    tc0 = PIX - cnt_t.sum(1, keepdims=True)
    tc_full = np.concatenate([tc0, cnt_t], axis=1)
    pcn = pc / (pc.sum(1, keepdims=True) + 1e-8)
    tcn = tc_full / (tc_full.sum(1, keepdims=True) + 1e-8)
    cbal = ((pcn - tcn) ** 2).mean() * 0.2

    def centers(cnt, sy, sx):
        d = np.maximum(cnt, 1.0)
        return sy / d, sx / d, cnt > 0

    cyp, cxp, prp = centers(cnt_p, sy_p, sx_p)
    cyt, cxt, prt = centers(cnt_t, sy_t, sx_t)
    PI, PJ = np.triu_indices(NCOL, 1)
    NP = PI.shape[0]

    def compact(cy, cx, pres):
        d = np.sqrt((cy[:, PI] - cy[:, PJ]) ** 2 + (cx[:, PI] - cx[:, PJ]) ** 2)
        vv = pres[:, PI] & pres[:, PJ]
        rank = np.cumsum(vv, axis=1) - 1
        slot = np.where(vv, rank, NP)
        comp = np.zeros((B, NP + 1))
        np.put_along_axis(comp, slot, d, axis=1)
        return comp[:, :NP], vv.sum(1)

    dpc, n_p = compact(cyp, cxp, prp)
    dtc, n_t = compact(cyt, cxt, prt)
    m = np.minimum(n_p, n_t)
    use = np.arange(NP)[None, :] < m[:, None]
    sq = (((dpc - dtc) ** 2) * use).sum(1)
    geo_b = np.where(m > 0, sq / np.maximum(m, 1), 0.0)
    geo = geo_b.sum() / B * 0.5

    total = (spatial_focal + transform + affine + rotation + reflection
             + geo + edge + cbal + exact_bonus)
    return np.array([total, spatial_focal, transform, exact_bonus, exact_count,
                     affine, rotation, reflection, geo, edge, cbal], np.float32)


def kernel(**inputs) -> np.ndarray:
    outs, _ = run_device(inputs, trace=False)
    return assemble(outs)


# revision 17
# speedup vs baseline: 1.3120x; 1.0028x over previous
"""AtlasSpecializedLoss on 8 TRN2 NeuronCores — pure data parallel over B.
v4: bf16 compute pipeline + fused DVE ops (tensor_tensor_reduce /
scalar_tensor_tensor), focal computed from softmax (no ptacc path),
per-color center stats via per-channel ttr accumulations (no big
rows/cols tensor_reduce), GpSimd only for odd-offset sobel/edge adds."""

import sys

for _p in ("/opt/trn_rl_repo", "/opt/pypackages"):
    if _p not in sys.path:
        sys.path.append(_p)

import numpy as np

import concourse.bass as bass
import concourse.bacc as bacc
from concourse import mybir
from concourse.tile import TileContext
from concourse.bass_utils import run_bass_kernel_spmd

F32 = mybir.dt.float32
BF16 = mybir.dt.bfloat16
AF = mybir.ActivationFunctionType
OP = mybir.AluOpType
AX = mybir.AxisListType

B, C, H, W = 4096, 10, 30, 30
PIX = H * W
NCOL = C - 1
NCORE = 8
BS = B // NCORE
P = 128
NT = BS // P
CH = 5

# out layout (f32 per row)
O_MPT, O_MCP, O_MC2, O_FOC, O_EDG, O_AFF, O_ROT, O_RFL = 0, 1, 2, 3, 4, 5, 6, 7
O_PC = 8            # 10
O_CNP, O_SYP, O_SXP = 18, 27, 36   # 9 each (colors 1..9) for pred argmax
O_CNT, O_SYT, O_SXT = 45, 54, 63   # 9 each for target
OUTW = 72


def _bc(ap, pos, n):
    dims = list(ap.ap)
    dims.insert(pos + 1, [0, n])
    return bass.AP(tensor=ap.tensor, offset=ap.offset, ap=dims)


def _ttr(v, out, in0, in1, op0, accum):
    """fused (in0 op0 in1) -> sum into accum [P,1], via scalar_tensor_tensor
    (out = (in0 op0' scalar) op1 in1 with accum_out)."""
    if op0 == OP.mult:
        v.scalar_tensor_tensor(out, in0, 1.0, in1, OP.mult, OP.mult,
                               accum_out=accum)
    else:
        v.scalar_tensor_tensor(out, in0, 0.0, in1, OP.bypass, op0,
                               accum_out=accum)


def build_graph() -> bass.Bass:
    nc = bacc.Bacc()
    pred = nc.declare_dram_parameter("pred", [BS, C * PIX], F32, isOutput=False)
    targ = nc.declare_dram_parameter("targ", [BS, C * PIX], F32, isOutput=False)
    ig = nc.declare_dram_parameter("ig", [BS, C * PIX], F32, isOutput=False)
    theta = nc.declare_dram_parameter("theta", [BS, 6], F32, isOutput=False)
    rot = nc.declare_dram_parameter("rot", [BS, 8], F32, isOutput=False)
    refl = nc.declare_dram_parameter("refl", [BS, 4], F32, isOutput=False)
    xmapd = nc.declare_dram_parameter("xmap", [1, PIX], BF16, isOutput=False)
    ymapd = nc.declare_dram_parameter("ymap", [1, PIX], BF16, isOutput=False)
    out = nc.declare_dram_parameter("out", [BS, OUTW], F32, isOutput=True)

    v = nc.vector
    a = nc.scalar
    g = nc.gpsimd

    with TileContext(nc) as tc:
        # pin combined ln+exp+copy+square act table (avoids per-switch reloads)
        atl = mybir.InstLoadActFuncSet(
            name=nc.get_next_instruction_name(), ins=[], outs=[])
        atl.act_func_set_id = 6
        nc.scalar.add_instruction(atl)
        with (
            tc.tile_pool(name="pr", bufs=1) as prp,
            tc.tile_pool(name="tg", bufs=1) as tgp,
            tc.tile_pool(name="igp", bufs=1) as igp,
            tc.tile_pool(name="eb", bufs=2) as ebp,      # prE / mp rotate here
            tc.tile_pool(name="tgb", bufs=1) as tgbp,
            tc.tile_pool(name="tf", bufs=1) as tfp,      # f32 tree scratch
            tc.tile_pool(name="tb", bufs=1) as tbp,      # bf16 big scratch
            tc.tile_pool(name="sb", bufs=3) as sbp,      # sobel scratch [P,2,900]
            tc.tile_pool(name="sob", bufs=2) as sobp,    # pidx|tidx home
            tc.tile_pool(name="sp", bufs=6) as spp,      # small [P,900] bf16 rotat.
            tc.tile_pool(name="px", bufs=1) as pxp,      # named persistent smalls
            tc.tile_pool(name="outp", bufs=2) as outp,
            tc.tile_pool(name="tiny", bufs=8) as tiny,
            tc.tile_pool(name="cst", bufs=1) as cst,
        ):
            xmap = cst.tile([P, PIX], BF16, tag="xmap")
            src = xmapd[0:1, :]
            nc.sync.dma_start(out=xmap, in_=bass.AP(
                tensor=src.tensor, offset=src.offset, ap=[[0, P], [1, PIX]]))
            ymap = cst.tile([P, PIX], BF16, tag="ymap")
            src = ymapd[0:1, :]
            nc.sync.dma_start(out=ymap, in_=bass.AP(
                tensor=src.tensor, offset=src.offset, ap=[[0, P], [1, PIX]]))

            for t in range(NT):
                r0 = t * P

                pr = prp.tile([P, C, PIX], F32, tag="pr")
                nc.sync.dma_start(
                    out=pr[:, 0:CH, :],
                    in_=pred[r0:r0 + P, :CH * PIX].rearrange("p (c x) -> p c x", c=CH))
                nc.sync.dma_start(
                    out=pr[:, CH:C, :],
                    in_=pred[r0:r0 + P, CH * PIX:].rearrange("p (c x) -> p c x", c=CH))

                ot = outp.tile([P, OUTW], F32, tag="ot")
                g.memset(ot, 0.0)

                # ---- target: load 2 groups, cast to bf16 ----
                tgb = tgbp.tile([P, C, PIX], BF16, tag="tgb")
                for cg in range(2):
                    c0 = cg * CH
                    tg_t = tgp.tile([P, CH, PIX], F32, tag="tg")
                    nc.sync.dma_start(
                        out=tg_t,
                        in_=targ[r0:r0 + P, c0 * PIX:(c0 + CH) * PIX].rearrange(
                            "p (c x) -> p c x", c=CH))
                    a.activation(tgb[:, c0:c0 + CH, :], tg_t, AF.Copy)

                # ---- pred softmax pieces ----
                prE = ebp.tile([P, C, PIX], BF16, tag="eb")
                a.activation(prE, pr, AF.Exp)
                prEf = prE.rearrange("p c x -> p (c x)")
                s_b = tbp.tile([P, CH * PIX], BF16, tag="tb")
                v.tensor_add(s_b, prEf[:, 0:4500], prEf[:, 4500:9000])
                v.tensor_add(s_b[:, 0:1800], s_b[:, 0:1800], s_b[:, 1800:3600])
                v.tensor_add(s_b[:, 0:900], s_b[:, 0:900], s_b[:, 900:1800])
                ss = pxp.tile([P, PIX], BF16, tag="ss")
                v.tensor_add(ss, s_b[:, 0:900], s_b[:, 3600:4500])
                lr = pxp.tile([P, PIX], BF16, tag="lr")
                a.activation(lr, ss, AF.Ln)
                rr = pxp.tile([P, PIX], BF16, tag="rr")
                a.activation(rr, lr, AF.Exp, bias=0.0, scale=-1.0)

                # q = sum_c targ*softmax_num = prE[tidx]  (exact: targ one-hot)
                q_b = tbp.tile([P, CH * PIX], BF16, tag="tb")
                tgbf = tgb.rearrange("p c x -> p (c x)")
                q = pxp.tile([P, PIX], BF16, tag="q")
                v.tensor_mul(q_b, tgbf[:, 0:4500], prEf[:, 0:4500])
                v.tensor_add(q_b[:, 0:1800], q_b[:, 0:1800], q_b[:, 1800:3600])
                v.tensor_add(q_b[:, 0:900], q_b[:, 0:900], q_b[:, 900:1800])
                v.tensor_add(q, q_b[:, 0:900], q_b[:, 3600:4500])
                q_b2 = tbp.tile([P, CH * PIX], BF16, tag="tb")
                v.tensor_mul(q_b2, tgbf[:, 4500:9000], prEf[:, 4500:9000])
                v.tensor_add(q_b2[:, 0:1800], q_b2[:, 0:1800], q_b2[:, 1800:3600])
                v.tensor_add(q_b2[:, 0:900], q_b2[:, 0:900], q_b2[:, 900:1800])
                v.tensor_add(q_b2[:, 0:900], q_b2[:, 0:900], q_b2[:, 3600:4500])
                v.tensor_add(q, q, q_b2[:, 0:900])

                # PC[c] = sum_pix softmax_c  (10 fused mult+reduce)
                pc_o = tbp.tile([P, CH * PIX], BF16, tag="tb")
                for c in range(C):
                    _ttr(v, pc_o[:, 0:900], prE[:, c, :], rr, OP.mult,
                         ot[:, O_PC + c:O_PC + c + 1])

                # ---- argmax over channels ----
                mf = tfp.tile([P, CH * PIX], F32, tag="tf")
                prf = pr.rearrange("p c x -> p (c x)")
                v.tensor_max(mf, prf[:, 0:4500], prf[:, 4500:9000])
                v.tensor_max(mf[:, 0:1800], mf[:, 0:1800], mf[:, 1800:3600])
                v.tensor_max(mf[:, 0:900], mf[:, 0:900], mf[:, 900:1800])
                mx = pxp.tile([P, PIX], F32, tag="mx")
                v.tensor_max(mx, mf[:, 0:900], mf[:, 3600:4500])
                mp = ebp.tile([P, C, PIX], BF16, tag="eb")
                v.tensor_tensor(mp, pr, _bc(mx, 0, C), OP.is_equal)

                # ---- copy-match: sum mp*ig  (2 fused mult+reduce) ----
                for cg in range(2):
                    c0 = cg * CH
                    ig_t = igp.tile([P, CH, PIX], F32, tag="ig")
                    nc.sync.dma_start(
                        out=ig_t,
                        in_=ig[r0:r0 + P, c0 * PIX:(c0 + CH) * PIX].rearrange(
                            "p (c x) -> p c x", c=CH))
                    igb = tbp.tile([P, CH * PIX], BF16, tag="tb")
                    a.activation(igb, ig_t.rearrange("p c x -> p (c x)"), AF.Copy)
                    slot = O_MCP if cg == 0 else O_MC2
                    _ttr(v, igb, mp[:, c0:c0 + CH, :].rearrange("p c x -> p (c x)"),
                         igb, OP.mult, ot[:, slot:slot + 1])

                # ---- pidx / tidx via weighted-channel stt chains ----
                sob = sobp.tile([P, 2, PIX], BF16, tag="sob")
                pidx = sob[:, 0, :]
                tidx = sob[:, 1, :]
                v.tensor_scalar(pidx, mp[:, 1, :], 1.0, None, OP.mult)
                for c in range(2, C):
                    v.scalar_tensor_tensor(pidx, mp[:, c, :], float(c), pidx,
                                           OP.mult, OP.add)
                v.tensor_scalar(tidx, tgb[:, 1, :], 1.0, None, OP.mult)
                for c in range(2, C):
                    v.scalar_tensor_tensor(tidx, tgb[:, c, :], float(c), tidx,
                                           OP.mult, OP.add)

                # exact-match count
                ex_o = spp.tile([P, PIX], BF16, tag="sp")
                _ttr(v, ex_o, pidx, tidx, OP.is_equal,
                     ot[:, O_MPT:O_MPT + 1])

                # ---- per-color center stats: cnt/sy/sx for mp and tgb ----
                st_o = tbp.tile([P, CH * PIX], BF16, tag="tb")
                for c in range(1, C):
                    v.tensor_scalar(st_o[:, 0:900], mp[:, c, :], 1.0, 0.0,
                                    OP.mult, OP.add,
                                    accum_out=ot[:, O_CNP + c - 1:O_CNP + c])
                    _ttr(v, st_o[:, 0:900], mp[:, c, :], ymap, OP.mult,
                         ot[:, O_SYP + c - 1:O_SYP + c])
                    _ttr(v, st_o[:, 0:900], mp[:, c, :], xmap, OP.mult,
                         ot[:, O_SXP + c - 1:O_SXP + c])
                for c in range(1, C):
                    v.tensor_scalar(st_o[:, 0:900], tgb[:, c, :], 1.0, 0.0,
                                    OP.mult, OP.add,
                                    accum_out=ot[:, O_CNT + c - 1:O_CNT + c])
                    _ttr(v, st_o[:, 0:900], tgb[:, c, :], ymap, OP.mult,
                         ot[:, O_SYT + c - 1:O_SYT + c])
                    _ttr(v, st_o[:, 0:900], tgb[:, c, :], xmap, OP.mult,
                         ot[:, O_SXT + c - 1:O_SXT + c])

                # ---- edges -> sw ----
                thw = tidx.rearrange("p (h w) -> p h w", w=W)
                ee = spp.tile([P, PIX], BF16, tag="sp")
                v.memset(ee, 0.0)
                dh = spp.tile([P, PIX], BF16, tag="sp")
                v.tensor_tensor(dh[:, :870], tidx[:, 30:], tidx[:, :870],
                                OP.not_equal)
                v.tensor_add(ee[:, 30:], ee[:, 30:], dh[:, :870])
                v.tensor_add(ee[:, :870], ee[:, :870], dh[:, :870])
                dw = spp.tile([P, PIX], BF16, tag="sp")
                dwv = dw[:, :870].rearrange("p (h w) -> p h w", w=29)
                v.tensor_tensor(dwv, thw[:, :, 1:], thw[:, :, :29], OP.not_equal)
                eehw = ee.rearrange("p (h w) -> p h w", w=W)
                v.tensor_add(eehw[:, :, 1:], eehw[:, :, 1:], dwv)
                v.tensor_add(eehw[:, :, :29], eehw[:, :, :29], dwv)
                sw = pxp.tile([P, PIX], BF16, tag="sw")
                v.tensor_scalar(sw, ee, 0.0, None, OP.is_gt)
                a.activation(sw, sw, AF.Copy, bias=1.0, scale=0.5)

                # ---- focal:  sum (1-pt)^1.4 * ln(pt) * sw   (host negates) ----
                pt = pxp.tile([P, PIX], BF16, tag="pt")
                v.tensor_mul(pt, q, rr)
                ceb = spp.tile([P, PIX], BF16, tag="sp")
                a.activation(ceb, pt, AF.Ln)
                tm = spp.tile([P, PIX], BF16, tag="sp")
                v.tensor_scalar(tm, pt, -1.0, 1.0, OP.mult, OP.add)
                v.tensor_scalar_max(tm, tm, 1e-30)
                a.activation(tm, tm, AF.Ln)
                vb = spp.tile([P, PIX], BF16, tag="sp")
                a.activation(vb, tm, AF.Exp, bias=0.0, scale=1.4)
                wf = spp.tile([P, PIX], BF16, tag="sp")
                v.tensor_mul(wf, vb, ceb)
                foc_o = spp.tile([P, PIX], BF16, tag="sp")
                _ttr(v, foc_o, wf, sw, OP.mult, ot[:, O_FOC:O_FOC + 1])

                # ---- sobel on [pidx|tidx] jointly ----
                S = sbp.tile([P, 2, PIX], BF16, tag="sb")
                v.tensor_scalar(S, sob, 2.0, None, OP.mult)
                v.tensor_add(S[:, :, 30:], S[:, :, 30:], sob[:, :, :870])
                v.tensor_add(S[:, :, :870], S[:, :, :870], sob[:, :, 30:])
                EX = sbp.tile([P, 2, PIX], BF16, tag="sb")
                vS = S.rearrange("p c (h w) -> p c h w", w=W)
                vE = EX.rearrange("p c (h w) -> p c h w", w=W)
                v.tensor_scalar(vE[:, :, :, 0:1], vS[:, :, :, 1:2], 1.0, None, OP.mult)
                v.tensor_scalar(vE[:, :, :, 29:30], vS[:, :, :, 28:29], -1.0, None, OP.mult)
                v.tensor_sub(vE[:, :, :, 1:29], vS[:, :, :, 2:], vS[:, :, :, :28])
                T = sbp.tile([P, 2, PIX], BF16, tag="sb")
                v.tensor_scalar(T, sob, 2.0, None, OP.mult)
                vI = sob.rearrange("p c (h w) -> p c h w", w=W)
                vT = T.rearrange("p c (h w) -> p c h w", w=W)
                v.tensor_add(vT[:, :, :, 1:], vT[:, :, :, 1:], vI[:, :, :, :29])
                v.tensor_add(vT[:, :, :, :29], vT[:, :, :, :29], vI[:, :, :, 1:])
                EY = sbp.tile([P, 2, PIX], BF16, tag="sb")
                v.tensor_scalar(EY[:, :, :30], T[:, :, 30:60], 1.0, None, OP.mult)
                v.tensor_scalar(EY[:, :, 870:], T[:, :, 840:870], -1.0, None, OP.mult)
                v.tensor_sub(EY[:, :, 30:870], T[:, :, 60:], T[:, :, :840])
                v.tensor_mul(EX, EX, EX)
                v.tensor_mul(EY, EY, EY)
                v.tensor_add(EX, EX, EY)
                v.tensor_scalar_max(EX, EX, 1e-30)
                a.activation(EX, EX, AF.Ln)
                a.activation(EX, EX, AF.Exp, bias=0.0, scale=0.5)
                dm = spp.tile([P, PIX], BF16, tag="sp")
                v.tensor_sub(dm, EX[:, 0, :], EX[:, 1, :])
                a.activation(dm, dm, AF.Square, accum_out=ot[:, O_EDG:O_EDG + 1])

                # ---- theta / entropies (same as v3) ----
                th = tiny.tile([P, 6], F32, tag="th")
                nc.sync.dma_start(out=th, in_=theta[r0:r0 + P, :])
                a.square(th, th)
                ssum = tiny.tile([P, 2], F32, tag="ssum")
                v.tensor_reduce(ssum[:, 0:1],
                                th.rearrange("p (r k) -> p r k", k=3)[:, :, 0:2],
                                axis=AX.XY, op=OP.add)
                v.tensor_reduce(ssum[:, 1:2],
                                th.rearrange("p (r k) -> p r k", k=3)[:, :, 2:3],
                                axis=AX.XY, op=OP.add)
                v.tensor_scalar_max(ssum, ssum, 1e-30)
                a.activation(ssum, ssum, AF.Ln)
                a.activation(ssum, ssum, AF.Exp, bias=0.0, scale=0.5)
                qq = tiny.tile([P, 1], F32, tag="q1")
                a.activation(qq, ssum[:, 1:2], AF.Copy, bias=0.0, scale=0.1)
                v.tensor_add(ot[:, O_AFF:O_AFF + 1], ssum[:, 0:1], qq)

                def entropy(src2, n, dst, tagp):
                    lgt = tiny.tile([P, n], F32, tag=tagp)
                    nc.sync.dma_start(out=lgt, in_=src2[r0:r0 + P, :])
                    m8 = tiny.tile([P, 1], F32, tag=tagp + "m")
                    v.tensor_reduce(m8, lgt, axis=AX.X, op=OP.max)
                    nm = tiny.tile([P, 1], F32, tag=tagp + "n")
                    a.activation(nm, m8, AF.Copy, bias=0.0, scale=-1.0)
                    z8 = tiny.tile([P, n], F32, tag=tagp + "z")
                    v.tensor_scalar(z8, lgt, nm, None, OP.add)
                    e8 = tiny.tile([P, n], F32, tag=tagp + "e")
                    a.activation(e8, lgt, AF.Exp, bias=nm)
                    s8 = tiny.tile([P, 1], F32, tag=tagp + "s")
                    v.tensor_reduce(s8, e8, axis=AX.X, op=OP.add)
                    dot = tiny.tile([P, 1], F32, tag=tagp + "d")
                    dsk = tiny.tile([P, n], F32, tag=tagp + "k")
                    v.tensor_mul(dsk, e8, z8)
                    v.tensor_reduce(dot, dsk, axis=AX.X, op=OP.add)
                    r8 = tiny.tile([P, 1], F32, tag=tagp + "r")
                    v.reciprocal(r8, s8)
                    v.tensor_mul(dot, dot, r8)
                    a.activation(s8, s8, AF.Ln)
                    v.tensor_sub(dst, s8, dot)

                entropy(rot, 8, ot[:, O_ROT:O_ROT + 1], "ro")
                entropy(refl, 4, ot[:, O_RFL:O_RFL + 1], "rf")

                nc.sync.dma_start(out=out[r0:r0 + P, :], in_=ot)
    nc.finalize()
    return nc


_GRAPH = None


def _get_graph():
    global _GRAPH
    if _GRAPH is None:
        _GRAPH = build_graph()
    return _GRAPH


def run_device(inputs: dict, trace: bool = False):
    pred = np.asarray(inputs["pred_output"], np.float32).reshape(B, C * PIX)
    targ = np.asarray(inputs["target_output"], np.float32).reshape(B, C * PIX)
    igrid = np.asarray(inputs["input_grid"], np.float32).reshape(B, C * PIX)
    theta = np.asarray(inputs["theta"], np.float32).reshape(B, 6)
    rot = np.asarray(inputs["rotation_logits"], np.float32).reshape(B, 8)
    refl = np.asarray(inputs["reflection_logits"], np.float32).reshape(B, 4)

    import ml_dtypes
    xmap = np.tile(np.arange(W, dtype=np.float32), H).reshape(1, PIX)
    ymap = np.repeat(np.arange(H, dtype=np.float32), W).reshape(1, PIX)
    xmap = xmap.astype(ml_dtypes.bfloat16)
    ymap = ymap.astype(ml_dtypes.bfloat16)

    in_maps = []
    for i in range(NCORE):
        s = slice(i * BS, (i + 1) * BS)
        in_maps.append({
            "pred": np.ascontiguousarray(pred[s]),
            "targ": np.ascontiguousarray(targ[s]),
            "ig": np.ascontiguousarray(igrid[s]),
            "theta": np.ascontiguousarray(theta[s]),
            "rot": np.ascontiguousarray(rot[s]),
            "refl": np.ascontiguousarray(refl[s]),
            "xmap": xmap,
            "ymap": ymap,
        })
    res = run_bass_kernel_spmd(_get_graph(), in_maps, core_ids=list(range(NCORE)),
                               trace=trace)
    outs = np.concatenate([r["out"] for r in res.results], axis=0)
    return outs, res


def assemble(outs: np.ndarray) -> np.ndarray:
    o = outs.astype(np.float64)
    npix = float(B * PIX)
    match_pt = o[:, O_MPT]
    match_cp = o[:, O_MCP] + o[:, O_MC2]
    spatial_focal = -o[:, O_FOC].sum() / npix
    exact = match_pt == PIX
    exact_count = exact.sum()
    exact_bonus = -exact.mean() * 7.0
    transform = (match_cp == PIX).mean() * 0.2
    affine = o[:, O_AFF].mean() * 0.4
    rotation = o[:, O_ROT].mean() * 0.3
    reflection = o[:, O_RFL].mean() * 0.3
    edge = o[:, O_EDG].sum() / npix * 0.3

    pc = o[:, O_PC:O_PC + C]
    cnt_p = o[:, O_CNP:O_CNP + NCOL]
    sy_p = o[:, O_SYP:O_SYP + NCOL]
    sx_p = o[:, O_SXP:O_SXP + NCOL]
    cnt_t = o[:, O_CNT:O_CNT + NCOL]
    sy_t = o[:, O_SYT:O_SYT + NCOL]
    sx_t = o[:, O_SXT:O_SXT + NCOL]

    tc0 = PIX - cnt_t.sum(1, keepdims=True)
    tc_full = np.concatenate([tc0, cnt_t], axis=1)
    pcn = pc / (pc.sum(1, keepdims=True) + 1e-8)
    tcn = tc_full / (tc_full.sum(1, keepdims=True) + 1e-8)
    cbal = ((pcn - tcn) ** 2).mean() * 0.2

    def centers(cnt, sy, sx):
        d = np.maximum(cnt, 1.0)
        return sy / d, sx / d, cnt > 0

    cyp, cxp, prp = centers(cnt_p, sy_p, sx_p)
    cyt, cxt, prt = centers(cnt_t, sy_t, sx_t)
    PI, PJ = np.triu_indices(NCOL, 1)
    NP = PI.shape[0]

    def compact(cy, cx, pres):
        d = np.sqrt((cy[:, PI] - cy[:, PJ]) ** 2 + (cx[:, PI] - cx[:, PJ]) ** 2)
        vv = pres[:, PI] & pres[:, PJ]
        rank = np.cumsum(vv, axis=1) - 1
        slot = np.where(vv, rank, NP)
        comp = np.zeros((B, NP + 1))
        np.put_along_axis(comp, slot, d, axis=1)
        return comp[:, :NP], vv.sum(1)

    dpc, n_p = compact(cyp, cxp, prp)
    dtc, n_t = compact(cyt, cxt, prt)
    m = np.minimum(n_p, n_t)
    use = np.arange(NP)[None, :] < m[:, None]
    sq = (((dpc - dtc) ** 2) * use).sum(1)
    geo_b = np.where(m > 0, sq / np.maximum(m, 1), 0.0)
    geo = geo_b.sum() / B * 0.5

    total = (spatial_focal + transform + affine + rotation + reflection
             + geo + edge + cbal + exact_bonus)
    return np.array([total, spatial_focal, transform, exact_bonus, exact_count,
                     affine, rotation, reflection, geo, edge, cbal], np.float32)


def kernel(**inputs) -> np.ndarray:
    outs, _ = run_device(inputs, trace=False)
    return assemble(outs)


# revision 20
# speedup vs baseline: 1.5115x; 1.1521x over previous
"""AtlasSpecializedLoss on 8 TRN2 NeuronCores — pure data parallel over B.
v4: bf16 compute pipeline + fused DVE ops (tensor_tensor_reduce /
scalar_tensor_tensor), focal computed from softmax (no ptacc path),
per-color center stats via per-channel ttr accumulations (no big
rows/cols tensor_reduce), GpSimd only for odd-offset sobel/edge adds."""

import sys

for _p in ("/opt/trn_rl_repo", "/opt/pypackages"):
    if _p not in sys.path:
        sys.path.append(_p)

import numpy as np

import concourse.bass as bass
import concourse.bacc as bacc
from concourse import mybir
from concourse.tile import TileContext
from concourse.bass_utils import run_bass_kernel_spmd

F32 = mybir.dt.float32
BF16 = mybir.dt.bfloat16
AF = mybir.ActivationFunctionType
OP = mybir.AluOpType
AX = mybir.AxisListType

B, C, H, W = 4096, 10, 30, 30
PIX = H * W
NCOL = C - 1
NCORE = 8
BS = B // NCORE
P = 128
NT = BS // P
CH = 5

# out layout (f32 per row)
O_MPT, O_MCP, O_MC2, O_FOC, O_EDG, O_AFF, O_ROT, O_RFL = 0, 1, 2, 3, 4, 5, 6, 7
O_PC = 8            # 10
O_CNP, O_SYP, O_SXP = 18, 27, 36   # 9 each (colors 1..9) for pred argmax
O_CNT, O_SYT, O_SXT = 45, 54, 63   # 9 each for target
OUTW = 72


def _bc(ap, pos, n):
    dims = list(ap.ap)
    dims.insert(pos + 1, [0, n])
    return bass.AP(tensor=ap.tensor, offset=ap.offset, ap=dims)


def _ttr(v, out, in0, in1, op0, accum):
    """fused (in0 op0 in1) -> sum into accum [P,1], via scalar_tensor_tensor
    (out = (in0 op0' scalar) op1 in1 with accum_out)."""
    if op0 == OP.mult:
        v.scalar_tensor_tensor(out, in0, 1.0, in1, OP.mult, OP.mult,
                               accum_out=accum)
    else:
        v.scalar_tensor_tensor(out, in0, 0.0, in1, OP.bypass, op0,
                               accum_out=accum)


def build_graph() -> bass.Bass:
    nc = bacc.Bacc()
    pred = nc.declare_dram_parameter("pred", [BS, C * PIX], F32, isOutput=False)
    targ = nc.declare_dram_parameter("targ", [BS, C * PIX], F32, isOutput=False)
    ig = nc.declare_dram_parameter("ig", [BS, C * PIX], F32, isOutput=False)
    theta = nc.declare_dram_parameter("theta", [BS, 6], F32, isOutput=False)
    rot = nc.declare_dram_parameter("rot", [BS, 8], F32, isOutput=False)
    refl = nc.declare_dram_parameter("refl", [BS, 4], F32, isOutput=False)
    xmapd = nc.declare_dram_parameter("xmap", [1, PIX], BF16, isOutput=False)
    ymapd = nc.declare_dram_parameter("ymap", [1, PIX], BF16, isOutput=False)
    out = nc.declare_dram_parameter("out", [BS, OUTW], F32, isOutput=True)

    v = nc.vector
    a = nc.scalar
    g = nc.gpsimd

    with TileContext(nc) as tc:
        # pin combined ln+exp+copy+square act table (avoids per-switch reloads)
        atl = mybir.InstLoadActFuncSet(
            name=nc.get_next_instruction_name(), ins=[], outs=[])
        atl.act_func_set_id = 6
        nc.scalar.add_instruction(atl)
        with (
            tc.tile_pool(name="pr", bufs=1) as prp,
            tc.tile_pool(name="tg", bufs=1) as tgp,
            tc.tile_pool(name="igp", bufs=1) as igp,
            tc.tile_pool(name="eb", bufs=2) as ebp,      # prE / mp rotate here
            tc.tile_pool(name="tgb", bufs=1) as tgbp,
            tc.tile_pool(name="tf", bufs=1) as tfp,      # f32 tree scratch
            tc.tile_pool(name="tb", bufs=1) as tbp,      # bf16 big scratch
            tc.tile_pool(name="sb", bufs=3) as sbp,      # sobel scratch [P,2,900]
            tc.tile_pool(name="sob", bufs=2) as sobp,    # pidx|tidx home
            tc.tile_pool(name="sp", bufs=6) as spp,      # small [P,900] bf16 rotat.
            tc.tile_pool(name="px", bufs=1) as pxp,      # named persistent smalls
            tc.tile_pool(name="outp", bufs=2) as outp,
            tc.tile_pool(name="tiny", bufs=8) as tiny,
            tc.tile_pool(name="cst", bufs=1) as cst,
        ):
            xmap = cst.tile([P, PIX], BF16, tag="xmap")
            src = xmapd[0:1, :]
            nc.sync.dma_start(out=xmap, in_=bass.AP(
                tensor=src.tensor, offset=src.offset, ap=[[0, P], [1, PIX]]))
            ymap = cst.tile([P, PIX], BF16, tag="ymap")
            src = ymapd[0:1, :]
            nc.sync.dma_start(out=ymap, in_=bass.AP(
                tensor=src.tensor, offset=src.offset, ap=[[0, P], [1, PIX]]))

            for t in range(NT):
                r0 = t * P

                pr = prp.tile([P, C, PIX], F32, tag="pr")
                nc.sync.dma_start(
                    out=pr[:, 0:CH, :],
                    in_=pred[r0:r0 + P, :CH * PIX].rearrange("p (c x) -> p c x", c=CH))
                nc.sync.dma_start(
                    out=pr[:, CH:C, :],
                    in_=pred[r0:r0 + P, CH * PIX:].rearrange("p (c x) -> p c x", c=CH))

                ot = outp.tile([P, OUTW], F32, tag="ot")
                g.memset(ot, 0.0)

                # ---- target: load 2 groups, cast to bf16 ----
                tgb = tgbp.tile([P, C, PIX], BF16, tag="tgb")
                for cg in range(2):
                    c0 = cg * CH
                    tg_t = tgp.tile([P, CH, PIX], F32, tag="tg")
                    nc.sync.dma_start(
                        out=tg_t,
                        in_=targ[r0:r0 + P, c0 * PIX:(c0 + CH) * PIX].rearrange(
                            "p (c x) -> p c x", c=CH))
                    a.activation(tgb[:, c0:c0 + CH, :], tg_t, AF.Copy)

                # ---- pred softmax pieces ----
                prE = ebp.tile([P, C, PIX], BF16, tag="eb")
                a.activation(prE, pr, AF.Exp)
                prEf = prE.rearrange("p c x -> p (c x)")
                s_b = tbp.tile([P, CH * PIX], BF16, tag="tb")
                v.tensor_add(s_b, prEf[:, 0:4500], prEf[:, 4500:9000])
                v.tensor_add(s_b[:, 0:1800], s_b[:, 0:1800], s_b[:, 1800:3600])
                v.tensor_add(s_b[:, 0:900], s_b[:, 0:900], s_b[:, 900:1800])
                ss = pxp.tile([P, PIX], BF16, tag="ss")
                v.tensor_add(ss, s_b[:, 0:900], s_b[:, 3600:4500])
                lr = pxp.tile([P, PIX], BF16, tag="lr")
                a.activation(lr, ss, AF.Ln)
                rr = pxp.tile([P, PIX], BF16, tag="rr")
                a.activation(rr, lr, AF.Exp, bias=0.0, scale=-1.0)

                # q = sum_c targ*softmax_num = prE[tidx]  (exact: targ one-hot)
                q_b = tbp.tile([P, CH * PIX], BF16, tag="tb")
                tgbf = tgb.rearrange("p c x -> p (c x)")
                q = pxp.tile([P, PIX], BF16, tag="q")
                v.tensor_mul(q_b, tgbf[:, 0:4500], prEf[:, 0:4500])
                v.tensor_add(q_b[:, 0:1800], q_b[:, 0:1800], q_b[:, 1800:3600])
                v.tensor_add(q_b[:, 0:900], q_b[:, 0:900], q_b[:, 900:1800])
                v.tensor_add(q, q_b[:, 0:900], q_b[:, 3600:4500])
                q_b2 = tbp.tile([P, CH * PIX], BF16, tag="tb")
                v.tensor_mul(q_b2, tgbf[:, 4500:9000], prEf[:, 4500:9000])
                v.tensor_add(q_b2[:, 0:1800], q_b2[:, 0:1800], q_b2[:, 1800:3600])
                v.tensor_add(q_b2[:, 0:900], q_b2[:, 0:900], q_b2[:, 900:1800])
                v.tensor_add(q_b2[:, 0:900], q_b2[:, 0:900], q_b2[:, 3600:4500])
                v.tensor_add(q, q, q_b2[:, 0:900])

                # PC[c] = sum_pix softmax_c  (10 fused mult+reduce)
                pc_o = tbp.tile([P, CH * PIX], BF16, tag="tb")
                for c in range(C):
                    _ttr(v, pc_o[:, 0:900], prE[:, c, :], rr, OP.mult,
                         ot[:, O_PC + c:O_PC + c + 1])

                # ---- argmax over channels ----
                mf = tfp.tile([P, CH * PIX], F32, tag="tf")
                prf = pr.rearrange("p c x -> p (c x)")
                v.tensor_max(mf, prf[:, 0:4500], prf[:, 4500:9000])
                v.tensor_max(mf[:, 0:1800], mf[:, 0:1800], mf[:, 1800:3600])
                v.tensor_max(mf[:, 0:900], mf[:, 0:900], mf[:, 900:1800])
                mx = pxp.tile([P, PIX], F32, tag="mx")
                v.tensor_max(mx, mf[:, 0:900], mf[:, 3600:4500])
                mp = ebp.tile([P, C, PIX], BF16, tag="eb")
                v.tensor_tensor(mp, pr, _bc(mx, 0, C), OP.is_equal)

                # ---- copy-match: sum mp*ig  (2 fused mult+reduce) ----
                for cg in range(2):
                    c0 = cg * CH
                    ig_t = igp.tile([P, CH, PIX], F32, tag="ig")
                    nc.sync.dma_start(
                        out=ig_t,
                        in_=ig[r0:r0 + P, c0 * PIX:(c0 + CH) * PIX].rearrange(
                            "p (c x) -> p c x", c=CH))
                    igb = tbp.tile([P, CH * PIX], BF16, tag="tb")
                    a.activation(igb, ig_t.rearrange("p c x -> p (c x)"), AF.Copy)
                    slot = O_MCP if cg == 0 else O_MC2
                    _ttr(v, igb, mp[:, c0:c0 + CH, :].rearrange("p c x -> p (c x)"),
                         igb, OP.mult, ot[:, slot:slot + 1])

                # ---- pidx / tidx via weighted-channel stt chains ----
                sob = sobp.tile([P, 2, PIX], BF16, tag="sob")
                pidx = sob[:, 0, :]
                tidx = sob[:, 1, :]
                v.tensor_scalar(pidx, mp[:, 1, :], 1.0, None, OP.mult)
                for c in range(2, C):
                    v.scalar_tensor_tensor(pidx, mp[:, c, :], float(c), pidx,
                                           OP.mult, OP.add)
                v.tensor_scalar(tidx, tgb[:, 1, :], 1.0, None, OP.mult)
                for c in range(2, C):
                    v.scalar_tensor_tensor(tidx, tgb[:, c, :], float(c), tidx,
                                           OP.mult, OP.add)

                # exact-match count
                ex_o = spp.tile([P, PIX], BF16, tag="sp")
                _ttr(v, ex_o, pidx, tidx, OP.is_equal,
                     ot[:, O_MPT:O_MPT + 1])

                # ---- per-color center stats: cnt/sy/sx for mp and tgb ----
                st_o = tbp.tile([P, CH * PIX], BF16, tag="tb")
                for c in range(1, C):
                    a.activation(lr, mp[:, c, :], AF.Copy,
                                 accum_out=ot[:, O_CNP + c - 1:O_CNP + c])
                    _ttr(v, st_o[:, 0:900], mp[:, c, :], ymap, OP.mult,
                         ot[:, O_SYP + c - 1:O_SYP + c])
                    _ttr(v, st_o[:, 0:900], mp[:, c, :], xmap, OP.mult,
                         ot[:, O_SXP + c - 1:O_SXP + c])
                for c in range(1, C):
                    a.activation(lr, tgb[:, c, :], AF.Copy,
                                 accum_out=ot[:, O_CNT + c - 1:O_CNT + c])
                    _ttr(v, st_o[:, 0:900], tgb[:, c, :], ymap, OP.mult,
                         ot[:, O_SYT + c - 1:O_SYT + c])
                    _ttr(v, st_o[:, 0:900], tgb[:, c, :], xmap, OP.mult,
                         ot[:, O_SXT + c - 1:O_SXT + c])

                # ---- edges -> sw ----
                thw = tidx.rearrange("p (h w) -> p h w", w=W)
                ee = spp.tile([P, PIX], BF16, tag="sp")
                v.memset(ee, 0.0)
                dh = spp.tile([P, PIX], BF16, tag="sp")
                v.tensor_tensor(dh[:, :870], tidx[:, 30:], tidx[:, :870],
                                OP.not_equal)
                v.tensor_add(ee[:, 30:], ee[:, 30:], dh[:, :870])
                v.tensor_add(ee[:, :870], ee[:, :870], dh[:, :870])
                dw = spp.tile([P, PIX], BF16, tag="sp")
                dwv = dw[:, :870].rearrange("p (h w) -> p h w", w=29)
                v.tensor_tensor(dwv, thw[:, :, 1:], thw[:, :, :29], OP.not_equal)
                eehw = ee.rearrange("p (h w) -> p h w", w=W)
                v.tensor_add(eehw[:, :, 1:], eehw[:, :, 1:], dwv)
                v.tensor_add(eehw[:, :, :29], eehw[:, :, :29], dwv)
                sw = pxp.tile([P, PIX], BF16, tag="sw")
                v.tensor_scalar(sw, ee, 0.0, None, OP.is_gt)
                a.activation(sw, sw, AF.Copy, bias=1.0, scale=0.5)

                # ---- focal:  sum (1-pt)^1.4 * ln(pt) * sw   (host negates) ----
                pt = pxp.tile([P, PIX], BF16, tag="pt")
                v.tensor_mul(pt, q, rr)
                ceb = spp.tile([P, PIX], BF16, tag="sp")
                a.activation(ceb, pt, AF.Ln)
                tm = spp.tile([P, PIX], BF16, tag="sp")
                v.tensor_scalar(tm, pt, -1.0, 1.0, OP.mult, OP.add)
                v.tensor_scalar_max(tm, tm, 1e-30)
                a.activation(tm, tm, AF.Ln)
                vb = spp.tile([P, PIX], BF16, tag="sp")
                a.activation(vb, tm, AF.Exp, bias=0.0, scale=1.4)
                wf = spp.tile([P, PIX], BF16, tag="sp")
                v.tensor_mul(wf, vb, ceb)
                foc_o = spp.tile([P, PIX], BF16, tag="sp")
                _ttr(v, foc_o, wf, sw, OP.mult, ot[:, O_FOC:O_FOC + 1])

                # ---- sobel on [pidx|tidx] jointly ----
                S = sbp.tile([P, 2, PIX], BF16, tag="sb")
                v.tensor_scalar(S, sob, 2.0, None, OP.mult)
                v.tensor_add(S[:, :, 30:], S[:, :, 30:], sob[:, :, :870])
                v.tensor_add(S[:, :, :870], S[:, :, :870], sob[:, :, 30:])
                EX = sbp.tile([P, 2, PIX], BF16, tag="sb")
                vS = S.rearrange("p c (h w) -> p c h w", w=W)
                vE = EX.rearrange("p c (h w) -> p c h w", w=W)
                v.tensor_scalar(vE[:, :, :, 0:1], vS[:, :, :, 1:2], 1.0, None, OP.mult)
                v.tensor_scalar(vE[:, :, :, 29:30], vS[:, :, :, 28:29], -1.0, None, OP.mult)
                v.tensor_sub(vE[:, :, :, 1:29], vS[:, :, :, 2:], vS[:, :, :, :28])
                T = sbp.tile([P, 2, PIX], BF16, tag="sb")
                v.tensor_scalar(T, sob, 2.0, None, OP.mult)
                vI = sob.rearrange("p c (h w) -> p c h w", w=W)
                vT = T.rearrange("p c (h w) -> p c h w", w=W)
                v.tensor_add(vT[:, :, :, 1:], vT[:, :, :, 1:], vI[:, :, :, :29])
                v.tensor_add(vT[:, :, :, :29], vT[:, :, :, :29], vI[:, :, :, 1:])
                EY = sbp.tile([P, 2, PIX], BF16, tag="sb")
                v.tensor_scalar(EY[:, :, :30], T[:, :, 30:60], 1.0, None, OP.mult)
                v.tensor_scalar(EY[:, :, 870:], T[:, :, 840:870], -1.0, None, OP.mult)
                v.tensor_sub(EY[:, :, 30:870], T[:, :, 60:], T[:, :, :840])
                a.square(EX, EX)
                a.square(EY, EY)
                v.tensor_add(EX, EX, EY)
                v.tensor_scalar_max(EX, EX, 1e-30)
                a.activation(EX, EX, AF.Ln)
                a.activation(EX, EX, AF.Exp, bias=0.0, scale=0.5)
                dm = spp.tile([P, PIX], BF16, tag="sp")
                v.tensor_sub(dm, EX[:, 0, :], EX[:, 1, :])
                a.activation(dm, dm, AF.Square, accum_out=ot[:, O_EDG:O_EDG + 1])

                # ---- theta / entropies (same as v3) ----
                th = tiny.tile([P, 6], F32, tag="th")
                nc.sync.dma_start(out=th, in_=theta[r0:r0 + P, :])
                a.square(th, th)
                ssum = tiny.tile([P, 2], F32, tag="ssum")
                v.tensor_reduce(ssum[:, 0:1],
                                th.rearrange("p (r k) -> p r k", k=3)[:, :, 0:2],
                                axis=AX.XY, op=OP.add)
                v.tensor_reduce(ssum[:, 1:2],
                                th.rearrange("p (r k) -> p r k", k=3)[:, :, 2:3],
                                axis=AX.XY, op=OP.add)
                v.tensor_scalar_max(ssum, ssum, 1e-30)
                a.activation(ssum, ssum, AF.Ln)
                a.activation(ssum, ssum, AF.Exp, bias=0.0, scale=0.5)
                qq = tiny.tile([P, 1], F32, tag="q1")
                a.activation(qq, ssum[:, 1:2], AF.Copy, bias=0.0, scale=0.1)
                v.tensor_add(ot[:, O_AFF:O_AFF + 1], ssum[:, 0:1], qq)

                def entropy(src2, n, dst, tagp):
                    lgt = tiny.tile([P, n], F32, tag=tagp)
                    nc.sync.dma_start(out=lgt, in_=src2[r0:r0 + P, :])
                    m8 = tiny.tile([P, 1], F32, tag=tagp + "m")
                    v.tensor_reduce(m8, lgt, axis=AX.X, op=OP.max)
                    nm = tiny.tile([P, 1], F32, tag=tagp + "n")
                    a.activation(nm, m8, AF.Copy, bias=0.0, scale=-1.0)
                    z8 = tiny.tile([P, n], F32, tag=tagp + "z")
                    v.tensor_scalar(z8, lgt, nm, None, OP.add)
                    e8 = tiny.tile([P, n], F32, tag=tagp + "e")
                    a.activation(e8, lgt, AF.Exp, bias=nm)
                    s8 = tiny.tile([P, 1], F32, tag=tagp + "s")
                    v.tensor_reduce(s8, e8, axis=AX.X, op=OP.add)
                    dot = tiny.tile([P, 1], F32, tag=tagp + "d")
                    dsk = tiny.tile([P, n], F32, tag=tagp + "k")
                    v.tensor_mul(dsk, e8, z8)
                    v.tensor_reduce(dot, dsk, axis=AX.X, op=OP.add)
                    r8 = tiny.tile([P, 1], F32, tag=tagp + "r")
                    v.reciprocal(r8, s8)
                    v.tensor_mul(dot, dot, r8)
                    a.activation(s8, s8, AF.Ln)
                    v.tensor_sub(dst, s8, dot)

                entropy(rot, 8, ot[:, O_ROT:O_ROT + 1], "ro")
                entropy(refl, 4, ot[:, O_RFL:O_RFL + 1], "rf")

                nc.sync.dma_start(out=out[r0:r0 + P, :], in_=ot)
    nc.finalize()
    return nc


_GRAPH = None


def _get_graph():
    global _GRAPH
    if _GRAPH is None:
        _GRAPH = build_graph()
    return _GRAPH


def run_device(inputs: dict, trace: bool = False):
    pred = np.asarray(inputs["pred_output"], np.float32).reshape(B, C * PIX)
    targ = np.asarray(inputs["target_output"], np.float32).reshape(B, C * PIX)
    igrid = np.asarray(inputs["input_grid"], np.float32).reshape(B, C * PIX)
    theta = np.asarray(inputs["theta"], np.float32).reshape(B, 6)
    rot = np.asarray(inputs["rotation_logits"], np.float32).reshape(B, 8)
    refl = np.asarray(inputs["reflection_logits"], np.float32).reshape(B, 4)

    import ml_dtypes
    xmap = np.tile(np.arange(W, dtype=np.float32), H).reshape(1, PIX)
    ymap = np.repeat(np.arange(H, dtype=np.float32), W).reshape(1, PIX)
    xmap = xmap.astype(ml_dtypes.bfloat16)
    ymap = ymap.astype(ml_dtypes.bfloat16)

    in_maps = []
    for i in range(NCORE):
        s = slice(i * BS, (i + 1) * BS)
        in_maps.append({
            "pred": np.ascontiguousarray(pred[s]),
            "targ": np.ascontiguousarray(targ[s]),
            "ig": np.ascontiguousarray(igrid[s]),
            "theta": np.ascontiguousarray(theta[s]),
            "rot": np.ascontiguousarray(rot[s]),
            "refl": np.ascontiguousarray(refl[s]),
            "xmap": xmap,
            "ymap": ymap,
        })
    res = run_bass_kernel_spmd(_get_graph(), in_maps, core_ids=list(range(NCORE)),
                               trace=trace)
    outs = np.concatenate([r["out"] for r in res.results], axis=0)
    return outs, res


def assemble(outs: np.ndarray) -> np.ndarray:
    o = outs.astype(np.float64)
    npix = float(B * PIX)
    match_pt = o[:, O_MPT]
    match_cp = o[:, O_MCP] + o[:, O_MC2]
    spatial_focal = -o[:, O_FOC].sum() / npix
    exact = match_pt == PIX
    exact_count = exact.sum()
    exact_bonus = -exact.mean() * 7.0
    transform = (match_cp == PIX).mean() * 0.2
    affine = o[:, O_AFF].mean() * 0.4
    rotation = o[:, O_ROT].mean() * 0.3
    reflection = o[:, O_RFL].mean() * 0.3
    edge = o[:, O_EDG].sum() / npix * 0.3

    pc = o[:, O_PC:O_PC + C]
    cnt_p = o[:, O_CNP:O_CNP + NCOL]
    sy_p = o[:, O_SYP:O_SYP + NCOL]
    sx_p = o[:, O_SXP:O_SXP + NCOL]
    cnt_t = o[:, O_CNT:O_CNT + NCOL]
    sy_t = o[:, O_SYT:O_SYT + NCOL]
    sx_t = o[:, O_SXT:O_SXT + NCOL]

    tc0 = PIX - cnt_t.sum(1, keepdims=True)
    tc_full = np.concatenate([tc0, cnt_t], axis=1)
    pcn = pc / (pc.sum(1, keepdims=True) + 1e-8)
    tcn = tc_full / (tc_full.sum(1, keepdims=True) + 1e-8)
    cbal = ((pcn - tcn) ** 2).mean() * 0.2

    def centers(cnt, sy, sx):
        d = np.maximum(cnt, 1.0)
        return sy / d, sx / d, cnt > 0

    cyp, cxp, prp = centers(cnt_p, sy_p, sx_p)
    cyt, cxt, prt = centers(cnt_t, sy_t, sx_t)
    PI, PJ = np.triu_indices(NCOL, 1)
    NP = PI.shape[0]

    def compact(cy, cx, pres):
        d = np.sqrt((cy[:, PI] - cy[:, PJ]) ** 2 + (cx[:, PI] - cx[:, PJ]) ** 2)
        vv = pres[:, PI] & pres[:, PJ]
        rank = np.cumsum(vv, axis=1) - 1
        slot = np.where(vv, rank, NP)
        comp = np.zeros((B, NP + 1))
        np.put_along_axis(comp, slot, d, axis=1)
        return comp[:, :NP], vv.sum(1)

    dpc, n_p = compact(cyp, cxp, prp)
    dtc, n_t = compact(cyt, cxt, prt)
    m = np.minimum(n_p, n_t)
    use = np.arange(NP)[None, :] < m[:, None]
    sq = (((dpc - dtc) ** 2) * use).sum(1)
    geo_b = np.where(m > 0, sq / np.maximum(m, 1), 0.0)
    geo = geo_b.sum() / B * 0.5

    total = (spatial_focal + transform + affine + rotation + reflection
             + geo + edge + cbal + exact_bonus)
    return np.array([total, spatial_focal, transform, exact_bonus, exact_count,
                     affine, rotation, reflection, geo, edge, cbal], np.float32)


def kernel(**inputs) -> np.ndarray:
    outs, _ = run_device(inputs, trace=False)
    return assemble(outs)
